# revision 6
# baseline (speedup 1.0000x reference)
"""Self-contained Trainium2 (Bass/Tile) kernel for the DeviceGAT problem.

Computes, on 8 NeuronCores, the GAT layer + LayerNorm + two decoder MLPs of
reference.py, exploiting the deterministic graph structure:
  - edges = dense local clique (1024 local nodes, no self edges)
            + local->remote pairs (i -> L+i)
            + remote->local pairs (L+i -> i)
            + self loops for all 2048 nodes
  - so each local dst d has in-edges from all 1024 local nodes (incl. itself
    via the self loop) plus remote node L+d; each remote dst L+i has in-edges
    {i, L+i}.

Sharding: destination rows are sharded 8 ways (128 local dst rows + the
matching 128 remote dst rows per core).  Node features / params replicated.
The dense per-core alpha block [128 d, 4 h, 1024 s] is written to HBM and the
host permutes it into the reference per-edge order.
"""

import os
import sys
import types
import numpy as np

# ---------------------------------------------------------------------------
# Problem constants (from the reference problem definition; deterministic).
L = 1024
N = 2 * L
MAPPED = 32
HID = 64
HEADS = 4
OUT = HID * HEADS        # 256
IN_DIM = 128
NEG_SLOPE = 0.2
EPS = 1e-5
NCORES = 8
SH = L // NCORES         # 128 dst rows per core


def _ensure_axon_hooks_stub():
    """run_bass_kernel_spmd(trace=True) imports antenv.axon_hooks; provide a
    graceful stub when the image lacks it so tracing degrades instead of
    crashing.  (Harness runs trace=False and never hits this, but be safe.)"""
    try:
        import antenv.axon_hooks  # noqa: F401
        return
    except Exception:
        pass
    try:
        import antenv
    except Exception:
        antenv = types.ModuleType("antenv")
        sys.modules["antenv"] = antenv
    mod = types.ModuleType("antenv.axon_hooks")
    mod._HOOK = None

    def set_axon_ntff_profile_hook(hook):
        mod._HOOK = hook

    def get_axon_ntff_profile_hook():
        if mod._HOOK is not None:
            return mod._HOOK
        # try to self-register against the axon PJRT .so if present
        so = "/opt/axon/libaxon_pjrt.so"
        if os.path.exists(so):
            import contextlib
            import ctypes

            try:
                lib = ctypes.CDLL(so)
            except OSError:
                return None
            if not hasattr(lib, "axon_start_nrt_profile"):
                return None
            lib.axon_start_nrt_profile.argtypes = [
                ctypes.POINTER(ctypes.c_int64),
                ctypes.c_size_t,
            ]
            lib.axon_start_nrt_profile.restype = ctypes.c_int64
            lib.axon_stop_nrt_profile.argtypes = [ctypes.c_char_p]
            lib.axon_stop_nrt_profile.restype = ctypes.c_int64

            @contextlib.contextmanager
            def _hook(output_dir, device_ids):
                import jax

                jax.devices()
                if device_ids:
                    ids = (ctypes.c_int64 * len(device_ids))(*device_ids)
                    rc = lib.axon_start_nrt_profile(ids, len(device_ids))
                else:
                    rc = lib.axon_start_nrt_profile(None, 0)
                if rc != 0:
                    raise RuntimeError(f"axon_start_nrt_profile rc={rc}")
                try:
                    yield
                finally:
                    n = lib.axon_stop_nrt_profile(str(output_dir).encode())
                    print(f"profile: {n} file(s) in {output_dir}", file=sys.stderr)

            mod._HOOK = _hook
            return mod._HOOK
        return None

    mod.set_axon_ntff_profile_hook = set_axon_ntff_profile_hook
    mod.get_axon_ntff_profile_hook = get_axon_ntff_profile_hook
    sys.modules["antenv.axon_hooks"] = mod


# ---------------------------------------------------------------------------
# Bass kernel builder


def _build_nc():
    from contextlib import ExitStack

    import concourse.bass as bass
    import concourse.tile as tile
    from concourse import bacc, mybir
    from concourse.masks import make_identity

    f32 = mybir.dt.float32
    AF = mybir.ActivationFunctionType

    nc = bacc.Bacc(
        trn_type="TRN2", target_bir_lowering=False, debug=False, num_devices=NCORES
    )

    # ---- I/O ----
    nfT = nc.dram_tensor("nfT", [MAPPED, N], f32, kind="ExternalInput")
    nfT_loc = nc.dram_tensor("nfT_loc", [MAPPED, SH], f32, kind="ExternalInput")
    nfT_rem = nc.dram_tensor("nfT_rem", [MAPPED, SH], f32, kind="ExternalInput")
    Wg = nc.dram_tensor("Wg", [MAPPED, OUT], f32, kind="ExternalInput")
    Wsd = nc.dram_tensor("Wsd", [MAPPED, 2 * HEADS], f32, kind="ExternalInput")
    bgat = nc.dram_tensor("bgat", [1, OUT], f32, kind="ExternalInput")
    decs = {}
    for tag in ("l", "r"):
        decs[tag] = {
            "W1": nc.dram_tensor(f"W1{tag}", [OUT, 128], f32, kind="ExternalInput"),
            "b1": nc.dram_tensor(f"b1{tag}", [1, 128], f32, kind="ExternalInput"),
            "W2": nc.dram_tensor(f"W2{tag}", [128, 64], f32, kind="ExternalInput"),
            "b2": nc.dram_tensor(f"b2{tag}", [1, 64], f32, kind="ExternalInput"),
            "W3": nc.dram_tensor(f"W3{tag}", [64, 128], f32, kind="ExternalInput"),
            "b3": nc.dram_tensor(f"b3{tag}", [1, 128], f32, kind="ExternalInput"),
        }

    alpha_d = nc.dram_tensor("alpha_d", [SH, HEADS * L], f32, kind="ExternalOutput")
    asmall = nc.dram_tensor("asmall", [SH, 12], f32, kind="ExternalOutput")
    rec_l = nc.dram_tensor("rec_l", [2 * SH, IN_DIM], f32, kind="ExternalOutput")
    rec_r = nc.dram_tensor("rec_r", [2 * SH, IN_DIM], f32, kind="ExternalOutput")

    NLOC_T = L // 128  # 8 local node tiles

    with tile.TileContext(nc) as tc, ExitStack() as ctx:
        consts = ctx.enter_context(tc.tile_pool(name="consts", bufs=1))
        big = ctx.enter_context(tc.tile_pool(name="big", bufs=2))
        persist = ctx.enter_context(tc.tile_pool(name="persist", bufs=1))
        small = ctx.enter_context(tc.tile_pool(name="small", bufs=2))
        psS = ctx.enter_context(tc.tile_pool(name="psS", bufs=1, space="PSUM"))
        psT = ctx.enter_context(tc.tile_pool(name="psT", bufs=2, space="PSUM"))
        psA = ctx.enter_context(tc.tile_pool(name="psA", bufs=1, space="PSUM"))
        psX = ctx.enter_context(tc.tile_pool(name="psX", bufs=2, space="PSUM"))

        # ---- load constants ----
        sb_nfT = consts.tile([MAPPED, N], f32)
        nc.sync.dma_start(sb_nfT[:], nfT[:])
        sb_nfT_loc = consts.tile([MAPPED, SH], f32)
        nc.sync.dma_start(sb_nfT_loc[:], nfT_loc[:])
        sb_nfT_rem = consts.tile([MAPPED, SH], f32)
        nc.sync.dma_start(sb_nfT_rem[:], nfT_rem[:])
        sb_Wg = consts.tile([MAPPED, OUT], f32)
        nc.sync.dma_start(sb_Wg[:], Wg[:])
        sb_Wsd = consts.tile([MAPPED, 2 * HEADS], f32)
        nc.sync.dma_start(sb_Wsd[:], Wsd[:])
        sb_bgat = consts.tile([1, OUT], f32)
        nc.sync.dma_start(sb_bgat[:], bgat[:])
        sbd = {}
        for tag in ("l", "r"):
            d = decs[tag]
            sbd[tag] = {
                "W1": consts.tile([128, 2, 128], f32, tag=f"W1{tag}", name=f"sbW1{tag}"),
                "b1": consts.tile([1, 128], f32, tag=f"b1{tag}", name=f"sbb1{tag}"),
                "W2": consts.tile([128, 64], f32, tag=f"W2{tag}", name=f"sbW2{tag}"),
                "b2": consts.tile([1, 64], f32, tag=f"b2{tag}", name=f"sbb2{tag}"),
                "W3": consts.tile([64, 128], f32, tag=f"W3{tag}", name=f"sbW3{tag}"),
                "b3": consts.tile([1, 128], f32, tag=f"b3{tag}", name=f"sbb3{tag}"),
            }
            nc.sync.dma_start(
                sbd[tag]["W1"][:], d["W1"].rearrange("(a p) n -> p a n", p=128)
            )
            for k in ("b1", "W2", "b2", "W3", "b3"):
                nc.sync.dma_start(sbd[tag][k][:], d[k][:])

        eps_col = consts.tile([128, 1], f32)
        nc.vector.memset(eps_col[:], EPS)
        ones_row = consts.tile([1, 128], f32)
        nc.vector.memset(ones_row[:], 1.0)
        # head-selector: sel4[k, h*128+m] = (k == h); lhsT slice per head gives
        # a k=4 matmul that broadcasts a_srcT[h, :] across 128 partitions
        sel4 = consts.tile([HEADS, HEADS * 128], f32)
        nc.gpsimd.memset(sel4[:], 0.0)
        sel4v = sel4.rearrange("p (a b) -> p a b", a=HEADS)
        nc.gpsimd.affine_select(
            out=sel4v,
            in_=sel4v,
            compare_op=mybir.AluOpType.not_equal,
            fill=1.0,
            base=0,
            pattern=[[-1, HEADS], [0, 128]],
            channel_multiplier=1,
        )
        ident = consts.tile([128, 128], f32)
        make_identity(nc, ident[:])

        # ---- projections on PE ----
        # x natural for all local node tiles + the core's remote/local shard
        x_loc = []
        for t in range(NLOC_T):
            ps = psX.tile([128, OUT], f32, tag="psx")
            nc.tensor.matmul(
                ps[:], sb_nfT[:, t * 128 : (t + 1) * 128], sb_Wg[:]
            )
            xt = persist.tile([128, OUT], f32, tag=f"xloc{t}")
            if t % 2 == 0:
                nc.scalar.copy(xt[:], ps[:])
            else:
                nc.vector.tensor_copy(xt[:], ps[:])
            x_loc.append(xt)
        ps = psX.tile([128, OUT], f32, tag="psx")
        nc.tensor.matmul(ps[:], sb_nfT_rem[:], sb_Wg[:])
        x_rem = persist.tile([128, OUT], f32, tag="xrem")
        nc.scalar.copy(x_rem[:], ps[:])
        ps = psX.tile([128, OUT], f32, tag="psx")
        nc.tensor.matmul(ps[:], sb_nfT_loc[:], sb_Wg[:])
        x_shard = persist.tile([128, OUT], f32, tag="xshard")
        nc.vector.tensor_copy(x_shard[:], ps[:])

        # a_srcT rows for all local nodes: [4, 1024]
        psr = psS.tile([128, 2, 512], f32, tag="psS")
        nc.tensor.matmul(psr[0:HEADS, 0, :], sb_Wsd[:, 0:HEADS], sb_nfT[:, 0:512])
        nc.tensor.matmul(psr[0:HEADS, 1, :], sb_Wsd[:, 0:HEADS], sb_nfT[:, 512:1024])
        sb_asrcT = consts.tile([HEADS, 2, 512], f32)
        nc.scalar.copy(sb_asrcT[:], psr[0:HEADS, :, :])

        # a_src/a_dst natural columns for the shard rows
        psc = psX.tile([128, OUT], f32, tag="psx")
        nc.tensor.matmul(psc[:, 0 : 2 * HEADS], sb_nfT_loc[:], sb_Wsd[:])
        A_loc = persist.tile([128, 2 * HEADS], f32, tag="aloc")
        nc.vector.tensor_copy(A_loc[:], psc[:, 0 : 2 * HEADS])
        psc = psX.tile([128, OUT], f32, tag="psx")
        nc.tensor.matmul(psc[:, 0 : 2 * HEADS], sb_nfT_rem[:], sb_Wsd[:])
        A_rem = persist.tile([128, 2 * HEADS], f32, tag="arem")
        nc.vector.tensor_copy(A_rem[:], psc[:, 0 : 2 * HEADS])

        # Bgat broadcast tile (for h_rem)
        psb = psX.tile([128, OUT], f32, tag="psx")
        nc.tensor.matmul(psb[:], ones_row[:], sb_bgat[:])
        sb_Bg = persist.tile([128, OUT], f32, tag="bg")
        nc.scalar.copy(sb_Bg[:], psb[:])

        # ---- special-edge scores: e_rl | e_lr | e_rr  [128, 12] ----
        E3 = persist.tile([128, 12], f32, tag="E3")
        nc.vector.tensor_add(E3[:, 0:4], A_rem[:, 0:4], A_loc[:, 4:8])
        nc.vector.tensor_add(E3[:, 4:8], A_loc[:, 0:4], A_rem[:, 4:8])
        nc.vector.tensor_add(E3[:, 8:12], A_rem[:, 0:4], A_rem[:, 4:8])
        LR3 = persist.tile([128, 12], f32, tag="LR3")
        nc.scalar.activation(LR3[:], E3[:], AF.Prelu, alpha=NEG_SLOPE)
        EXP3 = persist.tile([128, 12], f32, tag="EXP3")
        nc.scalar.activation(EXP3[:], LR3[:], AF.Exp)

        # remote-dst softmax (2 edges)
        asm = persist.tile([128, 12], f32, tag="asm")  # a_rl | a_lr | a_rr
        Zr = persist.tile([128, 4], f32, tag="Zr")
        nc.vector.tensor_add(Zr[:], EXP3[:, 4:8], EXP3[:, 8:12])
        Rr = persist.tile([128, 4], f32, tag="Rr")
        nc.vector.reciprocal(Rr[:], Zr[:])
        nc.vector.tensor_mul(asm[:, 4:8], EXP3[:, 4:8], Rr[:])
        nc.vector.tensor_mul(asm[:, 8:12], EXP3[:, 8:12], Rr[:])

        # h_rem pieces that only need a_lr/a_rr
        hrem_t1 = persist.tile([128, OUT], f32, tag="hrem_t1")
        hrem_t2 = persist.tile([128, OUT], f32, tag="hrem_t2")
        for h in range(HEADS):
            hs = slice(h * HID, (h + 1) * HID)
            nc.vector.tensor_scalar_mul(hrem_t1[:, hs], x_shard[:, hs], asm[:, 4 + h : 5 + h])
            nc.vector.tensor_scalar_mul(hrem_t2[:, hs], x_rem[:, hs], asm[:, 8 + h : 9 + h])
        h_rem = persist.tile([128, OUT], f32, tag="h_rem")
        nc.vector.tensor_add(h_rem[:], hrem_t1[:], hrem_t2[:])
        nc.vector.tensor_add(h_rem[:], h_rem[:], sb_Bg[:])

        # ---- dense attention per head ----
        alpha_sb = persist.tile([SH, HEADS, L], f32, tag="alpha")
        zrow = persist.tile([128, HEADS], f32, tag="zrow")
        Rcol = persist.tile([128, HEADS], f32, tag="rcol")
        extra = persist.tile([128, OUT], f32, tag="extra")

        # aggregation PSUM, bias-prefilled per head region
        psagg = psA.tile([128, OUT], f32, tag="agg")
        for h in range(HEADS):
            nc.tensor.matmul(
                psagg[:, h * HID : (h + 1) * HID],
                ones_row[:],
                sb_bgat[:, h * HID : (h + 1) * HID],
                start=True,
                stop=False,
            )

        alphaT = []
        for h in range(HEADS):
            psSh = psS.tile([128, 2, 512], f32, tag="psS")
            selh = sel4[:, h * 128 : (h + 1) * 128]
            nc.tensor.matmul(psSh[:, 0, :], selh, sb_asrcT[:, 0, :])
            nc.tensor.matmul(psSh[:, 1, :], selh, sb_asrcT[:, 1, :])
            e_h = big.tile([128, 2, 512], f32, tag="e")
            nc.scalar.activation(
                e_h[:], psSh[:], AF.Prelu, bias=A_loc[:, 4 + h : 5 + h], alpha=NEG_SLOPE
            )
            ex_h = big.tile([128, 1024], f32, tag="ex")
            nc.scalar.activation(
                ex_h[:],
                e_h.rearrange("p a b -> p (a b)"),
                AF.Exp,
                accum_out=zrow[:, h : h + 1],
            )
            # Z = rowsum + exp(e_extra);  R = 1/Z
            nc.vector.tensor_add(
                zrow[:, h : h + 1], zrow[:, h : h + 1], EXP3[:, h : h + 1]
            )
            nc.vector.reciprocal(Rcol[:, h : h + 1], zrow[:, h : h + 1])
            nc.vector.tensor_scalar_mul(
                alpha_sb[:, h, :], ex_h[:], Rcol[:, h : h + 1]
            )
            # alpha of remote->local edge + extra aggregation term
            nc.vector.tensor_scalar_mul(
                asm[:, h : h + 1], EXP3[:, h : h + 1], Rcol[:, h : h + 1]
            )
            nc.vector.tensor_scalar_mul(
                extra[:, h * HID : (h + 1) * HID],
                x_rem[:, h * HID : (h + 1) * HID],
                asm[:, h : h + 1],
            )
            # DMA this head's dense alpha out
            nc.sync.dma_start(alpha_d[:, h * L : (h + 1) * L], alpha_sb[:, h, :])

            # transpose alpha -> [s, d] tiles
            aT = persist.tile([128, NLOC_T, 128], f32, tag=f"alphaT{h}")
            for g in range(2):
                psTt = psT.tile([128, 4, 128], f32, tag="psT")
                for q in range(4):
                    t = g * 4 + q
                    nc.tensor.transpose(
                        psTt[:, q, :],
                        alpha_sb[:, h, t * 128 : (t + 1) * 128],
                        ident[:],
                    )
                if g == 0:
                    nc.scalar.copy(aT[:, 0:4, :], psTt[:])
                else:
                    nc.vector.tensor_copy(aT[:, 4:8, :], psTt[:])
            alphaT.append(aT)

            # aggregation matmuls for this head
            for t in range(NLOC_T):
                nc.tensor.matmul(
                    psagg[:, h * HID : (h + 1) * HID],
                    aT[:, t, :],
                    x_loc[t][:, h * HID : (h + 1) * HID],
                    start=False,
                    stop=(t == NLOC_T - 1),
                )

        # ---- h_loc = agg + extra ----
        h_loc = persist.tile([128, OUT], f32, tag="h_loc")
        nc.vector.tensor_add(h_loc[:], psagg[:], extra[:])

        # small alphas out
        nc.sync.dma_start(asmall[:], asm[:])

        # ---- layernorm helper ----
        BNS = nc.vector.BN_STATS_DIM
        BNA = nc.vector.BN_AGGR_DIM

        def layer_norm(x_t, width, tagp):
            st = small.tile([128, BNS], f32, tag="bnst")
            nc.vector.bn_stats(st[:], x_t[:, 0:width])
            mv = small.tile([128, BNA], f32, tag="bnmv")
            nc.vector.bn_aggr(mv[:], st[:])
            lnv = small.tile([128, 1], f32, tag="lnv")
            nc.scalar.activation(lnv[:], mv[:, 1:2], AF.Ln, bias=eps_col[:])
            rstd = small.tile([128, 1], f32, tag="rstd")
            nc.scalar.activation(rstd[:], lnv[:], AF.Exp, scale=-0.5)
            negmr = small.tile([128, 1], f32, tag="negmr")
            nc.vector.tensor_scalar(
                negmr[:], mv[:, 0:1], rstd[:], -1.0,
                op0=mybir.AluOpType.mult, op1=mybir.AluOpType.mult,
            )
            out_t = big.tile([128, width], f32, tag=tagp)
            nc.scalar.activation(
                out_t[:], x_t[:, 0:width], AF.Identity, bias=negmr[:], scale=rstd[:]
            )
            return out_t

        hn_loc = layer_norm(h_loc, OUT, "hn")
        hn_rem = layer_norm(h_rem, OUT, "hn")

        # ---- transpose normalized h for decoder matmuls ----
        # hT[a][:, nt, :]: feature block a (0:128 / 128:256), node tile nt
        hTa = persist.tile([128, 2, 128], f32, tag="hTa")
        hTb = persist.tile([128, 2, 128], f32, tag="hTb")
        for i, (src_t, nt) in enumerate(((hn_loc, 0), (hn_rem, 1))):
            psTt = psT.tile([128, 4, 128], f32, tag="psT")
            nc.tensor.transpose(psTt[:, 0, :], src_t[:, 0:128], ident[:])
            nc.tensor.transpose(psTt[:, 1, :], src_t[:, 128:256], ident[:])
            if i == 0:
                nc.scalar.copy(hTa[:, nt, :], psTt[:, 0, :])
                nc.vector.tensor_copy(hTb[:, nt, :], psTt[:, 1, :])
            else:
                nc.vector.tensor_copy(hTa[:, nt, :], psTt[:, 0, :])
                nc.scalar.copy(hTb[:, nt, :], psTt[:, 1, :])

        # ---- decoders ----
        rec_sb = {}
        for di, tag in enumerate(("l", "r")):
            p = sbd[tag]
            rec_t = persist.tile([128, 2, IN_DIM], f32, tag=f"rec{tag}")
            rec_sb[tag] = rec_t
            for nt in range(2):
                # layer 1: [*,256] @ [256,128] + b1, relu
                ps1 = psX.tile([128, OUT], f32, tag="psx")
                nc.tensor.matmul(
                    ps1[:, 0:128], ones_row[:], p["b1"][:], start=True, stop=False
                )
                nc.tensor.matmul(
                    ps1[:, 0:128], hTa[:, nt, :], p["W1"][:, 0, :],
                    start=False, stop=False,
                )
                nc.tensor.matmul(
                    ps1[:, 0:128], hTb[:, nt, :], p["W1"][:, 1, :],
                    start=False, stop=True,
                )
                r1 = big.tile([128, 128], f32, tag="r1")
                nc.scalar.activation(r1[:], ps1[:, 0:128], AF.Relu)
                n1 = layer_norm(r1, 128, "n1")
                psn = psT.tile([128, 4, 128], f32, tag="psT")
                nc.tensor.transpose(psn[:, 0, :], n1[:], ident[:])
                n1T = big.tile([128, 128], f32, tag="n1T")
                if nt == 0:
                    nc.scalar.copy(n1T[:], psn[:, 0, :])
                else:
                    nc.vector.tensor_copy(n1T[:], psn[:, 0, :])

                # layer 2: [*,128] @ [128,64] + b2, relu
                ps2 = psX.tile([128, OUT], f32, tag="psx")
                nc.tensor.matmul(
                    ps2[:, 0:64], ones_row[:], p["b2"][:], start=True, stop=False
                )
                nc.tensor.matmul(
                    ps2[:, 0:64], n1T[:], p["W2"][:], start=False, stop=True
                )
                r2 = big.tile([128, 64], f32, tag="r2")
                nc.scalar.activation(r2[:], ps2[:, 0:64], AF.Relu)
                n2 = layer_norm(r2, 64, "n2")
                psn2 = psT.tile([128, 4, 128], f32, tag="psT")
                nc.tensor.transpose(psn2[0:64, 0, :], n2[:, 0:64], ident[:])
                n2T = big.tile([64, 128], f32, tag="n2T")
                if nt == 0:
                    nc.vector.tensor_copy(n2T[:], psn2[0:64, 0, :])
                else:
                    nc.scalar.copy(n2T[:], psn2[0:64, 0, :])

                # layer 3: [*,64] @ [64,128] + b3
                ps3 = psX.tile([128, OUT], f32, tag="psx")
                nc.tensor.matmul(
                    ps3[:, 0:IN_DIM], ones_row[:], p["b3"][:], start=True, stop=False
                )
                nc.tensor.matmul(
                    ps3[:, 0:IN_DIM], n2T[:], p["W3"][:], start=False, stop=True
                )
                if (nt + di) % 2 == 0:
                    nc.scalar.copy(rec_t[:, nt, :], ps3[:, 0:IN_DIM])
                else:
                    nc.vector.tensor_copy(rec_t[:, nt, :], ps3[:, 0:IN_DIM])

        nc.sync.dma_start(rec_l.rearrange("(a p) n -> p a n", p=128), rec_sb["l"][:])
        nc.sync.dma_start(rec_r.rearrange("(a p) n -> p a n", p=128), rec_sb["r"][:])

    nc.compile()
    return nc


_NC_CACHE = {}


def _get_nc():
    if "nc" not in _NC_CACHE:
        _NC_CACHE["nc"] = _build_nc()
    return _NC_CACHE["nc"]


# ---------------------------------------------------------------------------
# Host side


def _expected_edge_index():
    local = np.arange(L)
    s = np.repeat(local, L)
    d = np.tile(local, L)
    m = s != d
    src = np.concatenate([s[m], local, L + local, np.arange(N)])
    dst = np.concatenate([d[m], L + local, local, np.arange(N)])
    return np.stack([src, dst])


def _np_reference_fallback(node_features, gat, norm, dec_local, dec_remote,
                           node_types, edge_index):
    """Pure-numpy replica of the reference; used only if the edge structure
    is not the expected deterministic pattern."""
    x = node_features @ gat["W"]
    xh = x.reshape(N, HEADS, HID)
    a_src = (xh * gat["att_src"]).sum(-1)
    a_dst = (xh * gat["att_dst"]).sum(-1)
    src, dst = edge_index[0], edge_index[1]
    e = a_src[src] + a_dst[dst]
    e = np.where(e >= 0, e, NEG_SLOPE * e)
    emax = np.full((N, HEADS), -np.inf, np.float32)
    np.maximum.at(emax, dst, e)
    ex = np.exp(e - emax[dst])
    zs = np.zeros((N, HEADS), np.float32)
    np.add.at(zs, dst, ex)
    alpha = ex / zs[dst]
    msg = xh[src] * alpha[:, :, None]
    out = np.zeros((N, HEADS, HID), np.float32)
    np.add.at(out, dst, msg)
    out = out.reshape(N, OUT) + gat["bias"]

    def ln(v, g, b):
        m = v.mean(-1, keepdims=True)
        var = ((v - m) ** 2).mean(-1, keepdims=True)
        return (v - m) / np.sqrt(var + EPS) * g + b

    h = ln(out, norm["g"], norm["b"])

    def dec(v, p):
        t = ln(np.maximum(v @ p["W1"] + p["b1"], 0), p["g1"], p["b1n"])
        t = ln(np.maximum(t @ p["W2"] + p["b2"], 0), p["g2"], p["b2n"])
        return t @ p["W3"] + p["b3"]

    rl = dec(h, dec_local)
    rr = dec(h, dec_remote)
    rec = np.where((node_types == 1)[:, None], rl, rr)
    return rec.astype(np.float32), edge_index, alpha.astype(np.float32)


def _to_np(v):
    return {k: _to_np(x) for k, x in v.items()} if isinstance(v, dict) else np.asarray(v)


def kernel(node_features, gat, norm, dec_local, dec_remote, node_types,
           edge_index, trace=False):
    _ensure_axon_hooks_stub()
    node_features = np.asarray(node_features, np.float32)
    gat, norm = _to_np(gat), _to_np(norm)
    dec_local, dec_remote = _to_np(dec_local), _to_np(dec_remote)
    node_types_np = np.asarray(node_types)
    edge_index_np = np.asarray(edge_index)

    if not np.array_equal(edge_index_np.astype(np.int64), _expected_edge_index()):
        return _np_reference_fallback(
            node_features, gat, norm, dec_local, dec_remote,
            node_types_np, edge_index_np,
        )

    from concourse.bass_utils import run_bass_kernel_spmd

    f32 = np.float32
    W = gat["W"].astype(f32)
    att_src = gat["att_src"].astype(f32)
    att_dst = gat["att_dst"].astype(f32)
    Wsrc = (W.reshape(MAPPED, HEADS, HID) * att_src[None]).sum(-1)
    Wdst = (W.reshape(MAPPED, HEADS, HID) * att_dst[None]).sum(-1)
    Wsd = np.ascontiguousarray(np.concatenate([Wsrc, Wdst], axis=1))
    nfT = np.ascontiguousarray(node_features.T)

    # fold LN affine params into the following linear layer
    def fold(dec):
        g, b = norm["g"].astype(f32), norm["b"].astype(f32)
        W1 = g[:, None] * dec["W1"]
        b1 = dec["b1"] + b @ dec["W1"]
        W2 = dec["g1"][:, None] * dec["W2"]
        b2 = dec["b2"] + dec["b1n"] @ dec["W2"]
        W3 = dec["g2"][:, None] * dec["W3"]
        b3 = dec["b3"] + dec["b2n"] @ dec["W3"]
        return (
            np.ascontiguousarray(W1, f32), b1.reshape(1, -1).astype(f32),
            np.ascontiguousarray(W2, f32), b2.reshape(1, -1).astype(f32),
            np.ascontiguousarray(W3, f32), b3.reshape(1, -1).astype(f32),
        )

    W1l, b1l, W2l, b2l, W3l, b3l = fold(dec_local)
    W1r, b1r, W2r, b2r, W3r, b3r = fold(dec_remote)

    shared = {
        "nfT": nfT, "Wg": np.ascontiguousarray(W), "Wsd": Wsd,
        "bgat": gat["bias"].reshape(1, -1).astype(f32),
        "W1l": W1l, "b1l": b1l, "W2l": W2l, "b2l": b2l, "W3l": W3l, "b3l": b3l,
        "W1r": W1r, "b1r": b1r, "W2r": W2r, "b2r": b2r, "W3r": W3r, "b3r": b3r,
    }
    in_maps = []
    for c in range(NCORES):
        sl = slice(c * SH, (c + 1) * SH)
        m = dict(shared)
        m["nfT_loc"] = np.ascontiguousarray(nfT[:, sl])
        m["nfT_rem"] = np.ascontiguousarray(nfT[:, L + c * SH : L + (c + 1) * SH])
        in_maps.append(m)

    nc = _get_nc()
    res = run_bass_kernel_spmd(nc, in_maps, core_ids=list(range(NCORES)), trace=trace)
    _NC_CACHE["last_results"] = res
    outs = res.results

    # ---- host unshard / assembly ----
    dense = np.concatenate(
        [outs[c]["alpha_d"].reshape(SH, HEADS, L) for c in range(NCORES)], axis=0
    )  # [d, h, s]
    a_rl = np.concatenate([outs[c]["asmall"][:, 0:4] for c in range(NCORES)], axis=0)
    a_lr = np.concatenate([outs[c]["asmall"][:, 4:8] for c in range(NCORES)], axis=0)
    a_rr = np.concatenate([outs[c]["asmall"][:, 8:12] for c in range(NCORES)], axis=0)

    # clique edges in (s, d) order, diag removed
    per_edge = np.ascontiguousarray(np.transpose(dense, (2, 0, 1))).reshape(
        L * L, HEADS
    )
    mask = ~np.eye(L, dtype=bool).reshape(-1)
    clique = per_edge[mask]
    idx = np.arange(L)
    self_local = dense[idx, :, idx]  # [L, H]
    alpha = np.concatenate(
        [clique, a_lr, a_rl, self_local, a_rr], axis=0
    ).astype(np.float32)

    rec_local = np.empty((N, IN_DIM), np.float32)
    rec_remote = np.empty((N, IN_DIM), np.float32)
    for c in range(NCORES):
        sl = slice(c * SH, (c + 1) * SH)
        slr = slice(L + c * SH, L + (c + 1) * SH)
        rec_local[sl] = outs[c]["rec_l"][0:SH]
        rec_local[slr] = outs[c]["rec_l"][SH : 2 * SH]
        rec_remote[sl] = outs[c]["rec_r"][0:SH]
        rec_remote[slr] = outs[c]["rec_r"][SH : 2 * SH]
    reconstructed = np.where((node_types_np == 1)[:, None], rec_local, rec_remote)

    return reconstructed.astype(np.float32), edge_index_np, alpha


# revision 7
# speedup vs baseline: 1.1692x; 1.1692x over previous
"""Self-contained Trainium2 (Bass/Tile) kernel for the DeviceGAT problem.

Computes, on 8 NeuronCores, the GAT layer + LayerNorm + two decoder MLPs of
reference.py, exploiting the deterministic graph structure:
  - edges = dense local clique (1024 local nodes, no self edges)
            + local->remote pairs (i -> L+i)
            + remote->local pairs (L+i -> i)
            + self loops for all 2048 nodes
  - so each local dst d has in-edges from all 1024 local nodes (incl. itself
    via the self loop) plus remote node L+d; each remote dst L+i has in-edges
    {i, L+i}.

Sharding: destination rows are sharded 8 ways (128 local dst rows + the
matching 128 remote dst rows per core).  Node features / params replicated.
The dense per-core alpha block [128 d, 4 h, 1024 s] is written to HBM and the
host permutes it into the reference per-edge order.
"""

import os
import sys
import types
import numpy as np

# ---------------------------------------------------------------------------
# Problem constants (from the reference problem definition; deterministic).
L = 1024
N = 2 * L
MAPPED = 32
HID = 64
HEADS = 4
OUT = HID * HEADS        # 256
IN_DIM = 128
NEG_SLOPE = 0.2
EPS = 1e-5
NCORES = 8
SH = L // NCORES         # 128 dst rows per core


def _ensure_axon_hooks_stub():
    """run_bass_kernel_spmd(trace=True) imports antenv.axon_hooks; provide a
    graceful stub when the image lacks it so tracing degrades instead of
    crashing.  (Harness runs trace=False and never hits this, but be safe.)"""
    try:
        import antenv.axon_hooks  # noqa: F401
        return
    except Exception:
        pass
    try:
        import antenv
    except Exception:
        antenv = types.ModuleType("antenv")
        sys.modules["antenv"] = antenv
    mod = types.ModuleType("antenv.axon_hooks")
    mod._HOOK = None

    def set_axon_ntff_profile_hook(hook):
        mod._HOOK = hook

    def get_axon_ntff_profile_hook():
        if mod._HOOK is not None:
            return mod._HOOK
        # try to self-register against the axon PJRT .so if present
        so = "/opt/axon/libaxon_pjrt.so"
        if os.path.exists(so):
            import contextlib
            import ctypes

            try:
                lib = ctypes.CDLL(so)
            except OSError:
                return None
            if not hasattr(lib, "axon_start_nrt_profile"):
                return None
            lib.axon_start_nrt_profile.argtypes = [
                ctypes.POINTER(ctypes.c_int64),
                ctypes.c_size_t,
            ]
            lib.axon_start_nrt_profile.restype = ctypes.c_int64
            lib.axon_stop_nrt_profile.argtypes = [ctypes.c_char_p]
            lib.axon_stop_nrt_profile.restype = ctypes.c_int64

            @contextlib.contextmanager
            def _hook(output_dir, device_ids):
                import jax

                jax.devices()
                if device_ids:
                    ids = (ctypes.c_int64 * len(device_ids))(*device_ids)
                    rc = lib.axon_start_nrt_profile(ids, len(device_ids))
                else:
                    rc = lib.axon_start_nrt_profile(None, 0)
                if rc != 0:
                    raise RuntimeError(f"axon_start_nrt_profile rc={rc}")
                try:
                    yield
                finally:
                    n = lib.axon_stop_nrt_profile(str(output_dir).encode())
                    print(f"profile: {n} file(s) in {output_dir}", file=sys.stderr)

            mod._HOOK = _hook
            return mod._HOOK
        return None

    mod.set_axon_ntff_profile_hook = set_axon_ntff_profile_hook
    mod.get_axon_ntff_profile_hook = get_axon_ntff_profile_hook
    sys.modules["antenv.axon_hooks"] = mod


# ---------------------------------------------------------------------------
# Bass kernel builder


def _build_nc():
    from contextlib import ExitStack

    import concourse.bass as bass
    import concourse.tile as tile
    from concourse import bacc, mybir
    from concourse.masks import make_identity

    f32 = mybir.dt.float32
    AF = mybir.ActivationFunctionType

    nc = bacc.Bacc(
        trn_type="TRN2", target_bir_lowering=False, debug=False, num_devices=NCORES
    )

    # ---- I/O ----
    nfT = nc.dram_tensor("nfT", [MAPPED, N], f32, kind="ExternalInput")
    nfT_loc = nc.dram_tensor("nfT_loc", [MAPPED, SH], f32, kind="ExternalInput")
    nfT_rem = nc.dram_tensor("nfT_rem", [MAPPED, SH], f32, kind="ExternalInput")
    Wg = nc.dram_tensor("Wg", [MAPPED, OUT], f32, kind="ExternalInput")
    Wsd = nc.dram_tensor("Wsd", [MAPPED, 2 * HEADS], f32, kind="ExternalInput")
    bgat = nc.dram_tensor("bgat", [1, OUT], f32, kind="ExternalInput")
    decs = {}
    for tag in ("l", "r"):
        decs[tag] = {
            "W1": nc.dram_tensor(f"W1{tag}", [OUT, 128], f32, kind="ExternalInput"),
            "b1": nc.dram_tensor(f"b1{tag}", [1, 128], f32, kind="ExternalInput"),
            "W2": nc.dram_tensor(f"W2{tag}", [128, 64], f32, kind="ExternalInput"),
            "b2": nc.dram_tensor(f"b2{tag}", [1, 64], f32, kind="ExternalInput"),
            "W3": nc.dram_tensor(f"W3{tag}", [64, 128], f32, kind="ExternalInput"),
            "b3": nc.dram_tensor(f"b3{tag}", [1, 128], f32, kind="ExternalInput"),
        }

    alpha_d = nc.dram_tensor("alpha_d", [SH, HEADS * L], f32, kind="ExternalOutput")
    asmall = nc.dram_tensor("asmall", [SH, 12], f32, kind="ExternalOutput")
    rec_l = nc.dram_tensor("rec_l", [2 * SH, IN_DIM], f32, kind="ExternalOutput")
    rec_r = nc.dram_tensor("rec_r", [2 * SH, IN_DIM], f32, kind="ExternalOutput")

    NLOC_T = L // 128  # 8 local node tiles

    with tile.TileContext(nc) as tc, ExitStack() as ctx:
        consts = ctx.enter_context(tc.tile_pool(name="consts", bufs=1))
        big = ctx.enter_context(tc.tile_pool(name="big", bufs=2))
        persist = ctx.enter_context(tc.tile_pool(name="persist", bufs=1))
        small = ctx.enter_context(tc.tile_pool(name="small", bufs=2))
        psS = ctx.enter_context(tc.tile_pool(name="psS", bufs=1, space="PSUM"))
        psT = ctx.enter_context(tc.tile_pool(name="psT", bufs=2, space="PSUM"))
        psA = ctx.enter_context(tc.tile_pool(name="psA", bufs=1, space="PSUM"))
        psX = ctx.enter_context(tc.tile_pool(name="psX", bufs=2, space="PSUM"))

        # ---- load constants ----
        sb_nfT = consts.tile([MAPPED, N], f32)
        nc.sync.dma_start(sb_nfT[:], nfT[:])
        sb_nfT_loc = consts.tile([MAPPED, SH], f32)
        nc.sync.dma_start(sb_nfT_loc[:], nfT_loc[:])
        sb_nfT_rem = consts.tile([MAPPED, SH], f32)
        nc.sync.dma_start(sb_nfT_rem[:], nfT_rem[:])
        sb_Wg = consts.tile([MAPPED, OUT], f32)
        nc.sync.dma_start(sb_Wg[:], Wg[:])
        sb_Wsd = consts.tile([MAPPED, 2 * HEADS], f32)
        nc.sync.dma_start(sb_Wsd[:], Wsd[:])
        sb_bgat = consts.tile([1, OUT], f32)
        nc.sync.dma_start(sb_bgat[:], bgat[:])
        sbd = {}
        for tag in ("l", "r"):
            d = decs[tag]
            sbd[tag] = {
                "W1": consts.tile([128, 2, 128], f32, tag=f"W1{tag}", name=f"sbW1{tag}"),
                "b1": consts.tile([1, 128], f32, tag=f"b1{tag}", name=f"sbb1{tag}"),
                "W2": consts.tile([128, 64], f32, tag=f"W2{tag}", name=f"sbW2{tag}"),
                "b2": consts.tile([1, 64], f32, tag=f"b2{tag}", name=f"sbb2{tag}"),
                "W3": consts.tile([64, 128], f32, tag=f"W3{tag}", name=f"sbW3{tag}"),
                "b3": consts.tile([1, 128], f32, tag=f"b3{tag}", name=f"sbb3{tag}"),
            }
            nc.sync.dma_start(
                sbd[tag]["W1"][:], d["W1"].rearrange("(a p) n -> p a n", p=128)
            )
            for k in ("b1", "W2", "b2", "W3", "b3"):
                nc.sync.dma_start(sbd[tag][k][:], d[k][:])

        eps_col = consts.tile([128, 1], f32)
        nc.vector.memset(eps_col[:], EPS)
        ones_row = consts.tile([1, 128], f32)
        nc.vector.memset(ones_row[:], 1.0)
        # head-selector: sel4[k, h*128+m] = (k == h); lhsT slice per head gives
        # a k=4 matmul that broadcasts a_srcT[h, :] across 128 partitions
        sel4 = consts.tile([HEADS, HEADS * 128], f32)
        nc.gpsimd.memset(sel4[:], 0.0)
        sel4v = sel4.rearrange("p (a b) -> p a b", a=HEADS)
        nc.gpsimd.affine_select(
            out=sel4v,
            in_=sel4v,
            compare_op=mybir.AluOpType.not_equal,
            fill=1.0,
            base=0,
            pattern=[[-1, HEADS], [0, 128]],
            channel_multiplier=1,
        )
        ident = consts.tile([128, 128], f32)
        make_identity(nc, ident[:])

        # ---- projections on PE ----
        # x natural for all local node tiles + the core's remote/local shard
        x_loc = []
        for t in range(NLOC_T):
            ps = psX.tile([128, OUT], f32, tag="psx")
            nc.tensor.matmul(
                ps[:], sb_nfT[:, t * 128 : (t + 1) * 128], sb_Wg[:]
            )
            xt = persist.tile([128, OUT], f32, tag=f"xloc{t}")
            if t % 2 == 0:
                nc.scalar.copy(xt[:], ps[:])
            else:
                nc.vector.tensor_copy(xt[:], ps[:])
            x_loc.append(xt)
        ps = psX.tile([128, OUT], f32, tag="psx")
        nc.tensor.matmul(ps[:], sb_nfT_rem[:], sb_Wg[:])
        x_rem = persist.tile([128, OUT], f32, tag="xrem")
        nc.scalar.copy(x_rem[:], ps[:])
        ps = psX.tile([128, OUT], f32, tag="psx")
        nc.tensor.matmul(ps[:], sb_nfT_loc[:], sb_Wg[:])
        x_shard = persist.tile([128, OUT], f32, tag="xshard")
        nc.vector.tensor_copy(x_shard[:], ps[:])

        # a_srcT rows for all local nodes: [4, 1024]
        psr = psS.tile([128, 2, 512], f32, tag="psS")
        nc.tensor.matmul(psr[0:HEADS, 0, :], sb_Wsd[:, 0:HEADS], sb_nfT[:, 0:512])
        nc.tensor.matmul(psr[0:HEADS, 1, :], sb_Wsd[:, 0:HEADS], sb_nfT[:, 512:1024])
        sb_asrcT = consts.tile([HEADS, 2, 512], f32)
        nc.scalar.copy(sb_asrcT[:], psr[0:HEADS, :, :])

        # a_src/a_dst natural columns for the shard rows
        psc = psX.tile([128, OUT], f32, tag="psx")
        nc.tensor.matmul(psc[:, 0 : 2 * HEADS], sb_nfT_loc[:], sb_Wsd[:])
        A_loc = persist.tile([128, 2 * HEADS], f32, tag="aloc")
        nc.vector.tensor_copy(A_loc[:], psc[:, 0 : 2 * HEADS])
        psc = psX.tile([128, OUT], f32, tag="psx")
        nc.tensor.matmul(psc[:, 0 : 2 * HEADS], sb_nfT_rem[:], sb_Wsd[:])
        A_rem = persist.tile([128, 2 * HEADS], f32, tag="arem")
        nc.vector.tensor_copy(A_rem[:], psc[:, 0 : 2 * HEADS])

        # Bgat broadcast tile (for h_rem)
        psb = psX.tile([128, OUT], f32, tag="psx")
        nc.tensor.matmul(psb[:], ones_row[:], sb_bgat[:])
        sb_Bg = persist.tile([128, OUT], f32, tag="bg")
        nc.scalar.copy(sb_Bg[:], psb[:])

        # ---- special-edge scores: e_rl | e_lr | e_rr  [128, 12] ----
        E3 = persist.tile([128, 12], f32, tag="E3")
        nc.vector.tensor_add(E3[:, 0:4], A_rem[:, 0:4], A_loc[:, 4:8])
        nc.vector.tensor_add(E3[:, 4:8], A_loc[:, 0:4], A_rem[:, 4:8])
        nc.vector.tensor_add(E3[:, 8:12], A_rem[:, 0:4], A_rem[:, 4:8])
        LR3 = persist.tile([128, 12], f32, tag="LR3")
        nc.scalar.activation(LR3[:], E3[:], AF.Prelu, alpha=NEG_SLOPE)
        EXP3 = persist.tile([128, 12], f32, tag="EXP3")
        nc.scalar.activation(EXP3[:], LR3[:], AF.Exp)

        # remote-dst softmax (2 edges)
        asm = persist.tile([128, 12], f32, tag="asm")  # a_rl | a_lr | a_rr
        Zr = persist.tile([128, 4], f32, tag="Zr")
        nc.vector.tensor_add(Zr[:], EXP3[:, 4:8], EXP3[:, 8:12])
        Rr = persist.tile([128, 4], f32, tag="Rr")
        nc.vector.reciprocal(Rr[:], Zr[:])
        nc.vector.tensor_mul(asm[:, 4:8], EXP3[:, 4:8], Rr[:])
        nc.vector.tensor_mul(asm[:, 8:12], EXP3[:, 8:12], Rr[:])

        # h_rem pieces that only need a_lr/a_rr
        hrem_t1 = persist.tile([128, OUT], f32, tag="hrem_t1")
        hrem_t2 = persist.tile([128, OUT], f32, tag="hrem_t2")
        for h in range(HEADS):
            hs = slice(h * HID, (h + 1) * HID)
            nc.vector.tensor_scalar_mul(hrem_t1[:, hs], x_shard[:, hs], asm[:, 4 + h : 5 + h])
            nc.vector.tensor_scalar_mul(hrem_t2[:, hs], x_rem[:, hs], asm[:, 8 + h : 9 + h])
        h_rem = persist.tile([128, OUT], f32, tag="h_rem")
        nc.vector.tensor_add(h_rem[:], hrem_t1[:], hrem_t2[:])
        nc.vector.tensor_add(h_rem[:], h_rem[:], sb_Bg[:])

        # ---- dense attention per head ----
        alpha_sb = persist.tile([SH, HEADS, L], f32, tag="alpha")
        zrow = persist.tile([128, HEADS], f32, tag="zrow")
        Rcol = persist.tile([128, HEADS], f32, tag="rcol")
        extra = persist.tile([128, OUT], f32, tag="extra")

        # aggregation PSUM, bias-prefilled per head region
        psagg = psA.tile([128, OUT], f32, tag="agg")
        for h in range(HEADS):
            nc.tensor.matmul(
                psagg[:, h * HID : (h + 1) * HID],
                ones_row[:],
                sb_bgat[:, h * HID : (h + 1) * HID],
                start=True,
                stop=False,
            )

        alphaT = []
        for h in range(HEADS):
            psSh = psS.tile([128, 2, 512], f32, tag="psS")
            selh = sel4[:, h * 128 : (h + 1) * 128]
            nc.tensor.matmul(psSh[:, 0, :], selh, sb_asrcT[:, 0, :])
            nc.tensor.matmul(psSh[:, 1, :], selh, sb_asrcT[:, 1, :])
            e_h = big.tile([128, 2, 512], f32, tag="e")
            nc.scalar.activation(
                e_h[:], psSh[:], AF.Prelu, bias=A_loc[:, 4 + h : 5 + h], alpha=NEG_SLOPE
            )
            ex_h = big.tile([128, 1024], f32, tag="ex")
            nc.scalar.activation(
                ex_h[:],
                e_h.rearrange("p a b -> p (a b)"),
                AF.Exp,
                accum_out=zrow[:, h : h + 1],
            )
            # Z = rowsum + exp(e_extra);  R = 1/Z
            nc.vector.tensor_add(
                zrow[:, h : h + 1], zrow[:, h : h + 1], EXP3[:, h : h + 1]
            )
            nc.vector.reciprocal(Rcol[:, h : h + 1], zrow[:, h : h + 1])
            nc.vector.tensor_scalar_mul(
                alpha_sb[:, h, :], ex_h[:], Rcol[:, h : h + 1]
            )
            # alpha of remote->local edge + extra aggregation term
            nc.vector.tensor_scalar_mul(
                asm[:, h : h + 1], EXP3[:, h : h + 1], Rcol[:, h : h + 1]
            )
            nc.vector.tensor_scalar_mul(
                extra[:, h * HID : (h + 1) * HID],
                x_rem[:, h * HID : (h + 1) * HID],
                asm[:, h : h + 1],
            )
            # DMA this head's dense alpha out
            nc.sync.dma_start(alpha_d[:, h * L : (h + 1) * L], alpha_sb[:, h, :])

            # transpose alpha -> [s, d] tiles
            aT = persist.tile([128, NLOC_T, 128], f32, tag=f"alphaT{h}")
            for g in range(2):
                psTt = psT.tile([128, 4, 128], f32, tag="psT")
                for q in range(4):
                    t = g * 4 + q
                    nc.tensor.transpose(
                        psTt[:, q, :],
                        alpha_sb[:, h, t * 128 : (t + 1) * 128],
                        ident[:],
                    )
                if g == 0:
                    nc.scalar.copy(aT[:, 0:4, :], psTt[:])
                else:
                    nc.vector.tensor_copy(aT[:, 4:8, :], psTt[:])
            alphaT.append(aT)

            # aggregation matmuls for this head
            for t in range(NLOC_T):
                nc.tensor.matmul(
                    psagg[:, h * HID : (h + 1) * HID],
                    aT[:, t, :],
                    x_loc[t][:, h * HID : (h + 1) * HID],
                    start=False,
                    stop=(t == NLOC_T - 1),
                )

        # ---- h_loc = agg + extra ----
        h_loc = persist.tile([128, OUT], f32, tag="h_loc")
        nc.vector.tensor_add(h_loc[:], psagg[:], extra[:])

        # small alphas out
        nc.sync.dma_start(asmall[:], asm[:])

        # ---- layernorm helper ----
        BNS = nc.vector.BN_STATS_DIM
        BNA = nc.vector.BN_AGGR_DIM

        def layer_norm(x_t, width, tagp):
            st = small.tile([128, BNS], f32, tag="bnst")
            nc.vector.bn_stats(st[:], x_t[:, 0:width])
            mv = small.tile([128, BNA], f32, tag="bnmv")
            nc.vector.bn_aggr(mv[:], st[:])
            sdev = small.tile([128, 1], f32, tag="sdev")
            nc.scalar.activation(sdev[:], mv[:, 1:2], AF.Sqrt, bias=eps_col[:])
            rstd = small.tile([128, 1], f32, tag="rstd")
            nc.vector.reciprocal(rstd[:], sdev[:])
            negmr = small.tile([128, 1], f32, tag="negmr")
            nc.vector.tensor_scalar(
                negmr[:], mv[:, 0:1], rstd[:], -1.0,
                op0=mybir.AluOpType.mult, op1=mybir.AluOpType.mult,
            )
            out_t = big.tile([128, width], f32, tag=tagp)
            nc.scalar.activation(
                out_t[:], x_t[:, 0:width], AF.Identity, bias=negmr[:], scale=rstd[:]
            )
            return out_t

        hn_loc = layer_norm(h_loc, OUT, "hn")
        hn_rem = layer_norm(h_rem, OUT, "hn")

        # ---- transpose normalized h for decoder matmuls ----
        # hT[a][:, nt, :]: feature block a (0:128 / 128:256), node tile nt
        hTa = persist.tile([128, 2, 128], f32, tag="hTa")
        hTb = persist.tile([128, 2, 128], f32, tag="hTb")
        for i, (src_t, nt) in enumerate(((hn_loc, 0), (hn_rem, 1))):
            psTt = psT.tile([128, 4, 128], f32, tag="psT")
            nc.tensor.transpose(psTt[:, 0, :], src_t[:, 0:128], ident[:])
            nc.tensor.transpose(psTt[:, 1, :], src_t[:, 128:256], ident[:])
            if i == 0:
                nc.scalar.copy(hTa[:, nt, :], psTt[:, 0, :])
                nc.vector.tensor_copy(hTb[:, nt, :], psTt[:, 1, :])
            else:
                nc.vector.tensor_copy(hTa[:, nt, :], psTt[:, 0, :])
                nc.scalar.copy(hTb[:, nt, :], psTt[:, 1, :])

        # ---- decoders ----
        rec_sb = {}
        for di, tag in enumerate(("l", "r")):
            p = sbd[tag]
            rec_t = persist.tile([128, 2, IN_DIM], f32, tag=f"rec{tag}")
            rec_sb[tag] = rec_t
            for nt in range(2):
                # layer 1: [*,256] @ [256,128] + b1, relu
                ps1 = psX.tile([128, OUT], f32, tag="psx")
                nc.tensor.matmul(
                    ps1[:, 0:128], ones_row[:], p["b1"][:], start=True, stop=False
                )
                nc.tensor.matmul(
                    ps1[:, 0:128], hTa[:, nt, :], p["W1"][:, 0, :],
                    start=False, stop=False,
                )
                nc.tensor.matmul(
                    ps1[:, 0:128], hTb[:, nt, :], p["W1"][:, 1, :],
                    start=False, stop=True,
                )
                r1 = big.tile([128, 128], f32, tag="r1")
                nc.scalar.activation(r1[:], ps1[:, 0:128], AF.Relu)
                n1 = layer_norm(r1, 128, "n1")
                psn = psT.tile([128, 4, 128], f32, tag="psT")
                nc.tensor.transpose(psn[:, 0, :], n1[:], ident[:])
                n1T = big.tile([128, 128], f32, tag="n1T")
                if nt == 0:
                    nc.scalar.copy(n1T[:], psn[:, 0, :])
                else:
                    nc.vector.tensor_copy(n1T[:], psn[:, 0, :])

                # layer 2: [*,128] @ [128,64] + b2, relu
                ps2 = psX.tile([128, OUT], f32, tag="psx")
                nc.tensor.matmul(
                    ps2[:, 0:64], ones_row[:], p["b2"][:], start=True, stop=False
                )
                nc.tensor.matmul(
                    ps2[:, 0:64], n1T[:], p["W2"][:], start=False, stop=True
                )
                r2 = big.tile([128, 64], f32, tag="r2")
                nc.scalar.activation(r2[:], ps2[:, 0:64], AF.Relu)
                n2 = layer_norm(r2, 64, "n2")
                psn2 = psT.tile([128, 4, 128], f32, tag="psT")
                nc.tensor.transpose(psn2[0:64, 0, :], n2[:, 0:64], ident[:])
                n2T = big.tile([64, 128], f32, tag="n2T")
                if nt == 0:
                    nc.vector.tensor_copy(n2T[:], psn2[0:64, 0, :])
                else:
                    nc.scalar.copy(n2T[:], psn2[0:64, 0, :])

                # layer 3: [*,64] @ [64,128] + b3
                ps3 = psX.tile([128, OUT], f32, tag="psx")
                nc.tensor.matmul(
                    ps3[:, 0:IN_DIM], ones_row[:], p["b3"][:], start=True, stop=False
                )
                nc.tensor.matmul(
                    ps3[:, 0:IN_DIM], n2T[:], p["W3"][:], start=False, stop=True
                )
                if (nt + di) % 2 == 0:
                    nc.scalar.copy(rec_t[:, nt, :], ps3[:, 0:IN_DIM])
                else:
                    nc.vector.tensor_copy(rec_t[:, nt, :], ps3[:, 0:IN_DIM])

        nc.sync.dma_start(rec_l.rearrange("(a p) n -> p a n", p=128), rec_sb["l"][:])
        nc.sync.dma_start(rec_r.rearrange("(a p) n -> p a n", p=128), rec_sb["r"][:])

    nc.compile()
    return nc


_NC_CACHE = {}


def _get_nc():
    if "nc" not in _NC_CACHE:
        _NC_CACHE["nc"] = _build_nc()
    return _NC_CACHE["nc"]


# ---------------------------------------------------------------------------
# Host side


def _expected_edge_index():
    local = np.arange(L)
    s = np.repeat(local, L)
    d = np.tile(local, L)
    m = s != d
    src = np.concatenate([s[m], local, L + local, np.arange(N)])
    dst = np.concatenate([d[m], L + local, local, np.arange(N)])
    return np.stack([src, dst])


def _np_reference_fallback(node_features, gat, norm, dec_local, dec_remote,
                           node_types, edge_index):
    """Pure-numpy replica of the reference; used only if the edge structure
    is not the expected deterministic pattern."""
    x = node_features @ gat["W"]
    xh = x.reshape(N, HEADS, HID)
    a_src = (xh * gat["att_src"]).sum(-1)
    a_dst = (xh * gat["att_dst"]).sum(-1)
    src, dst = edge_index[0], edge_index[1]
    e = a_src[src] + a_dst[dst]
    e = np.where(e >= 0, e, NEG_SLOPE * e)
    emax = np.full((N, HEADS), -np.inf, np.float32)
    np.maximum.at(emax, dst, e)
    ex = np.exp(e - emax[dst])
    zs = np.zeros((N, HEADS), np.float32)
    np.add.at(zs, dst, ex)
    alpha = ex / zs[dst]
    msg = xh[src] * alpha[:, :, None]
    out = np.zeros((N, HEADS, HID), np.float32)
    np.add.at(out, dst, msg)
    out = out.reshape(N, OUT) + gat["bias"]

    def ln(v, g, b):
        m = v.mean(-1, keepdims=True)
        var = ((v - m) ** 2).mean(-1, keepdims=True)
        return (v - m) / np.sqrt(var + EPS) * g + b

    h = ln(out, norm["g"], norm["b"])

    def dec(v, p):
        t = ln(np.maximum(v @ p["W1"] + p["b1"], 0), p["g1"], p["b1n"])
        t = ln(np.maximum(t @ p["W2"] + p["b2"], 0), p["g2"], p["b2n"])
        return t @ p["W3"] + p["b3"]

    rl = dec(h, dec_local)
    rr = dec(h, dec_remote)
    rec = np.where((node_types == 1)[:, None], rl, rr)
    return rec.astype(np.float32), edge_index, alpha.astype(np.float32)


def _to_np(v):
    return {k: _to_np(x) for k, x in v.items()} if isinstance(v, dict) else np.asarray(v)


def kernel(node_features, gat, norm, dec_local, dec_remote, node_types,
           edge_index, trace=False):
    _ensure_axon_hooks_stub()
    node_features = np.asarray(node_features, np.float32)
    gat, norm = _to_np(gat), _to_np(norm)
    dec_local, dec_remote = _to_np(dec_local), _to_np(dec_remote)
    node_types_np = np.asarray(node_types)
    edge_index_np = np.asarray(edge_index)

    if not np.array_equal(edge_index_np.astype(np.int64), _expected_edge_index()):
        return _np_reference_fallback(
            node_features, gat, norm, dec_local, dec_remote,
            node_types_np, edge_index_np,
        )

    from concourse.bass_utils import run_bass_kernel_spmd

    f32 = np.float32
    W = gat["W"].astype(f32)
    att_src = gat["att_src"].astype(f32)
    att_dst = gat["att_dst"].astype(f32)
    Wsrc = (W.reshape(MAPPED, HEADS, HID) * att_src[None]).sum(-1)
    Wdst = (W.reshape(MAPPED, HEADS, HID) * att_dst[None]).sum(-1)
    Wsd = np.ascontiguousarray(np.concatenate([Wsrc, Wdst], axis=1))
    nfT = np.ascontiguousarray(node_features.T)

    # fold LN affine params into the following linear layer
    def fold(dec):
        g, b = norm["g"].astype(f32), norm["b"].astype(f32)
        W1 = g[:, None] * dec["W1"]
        b1 = dec["b1"] + b @ dec["W1"]
        W2 = dec["g1"][:, None] * dec["W2"]
        b2 = dec["b2"] + dec["b1n"] @ dec["W2"]
        W3 = dec["g2"][:, None] * dec["W3"]
        b3 = dec["b3"] + dec["b2n"] @ dec["W3"]
        return (
            np.ascontiguousarray(W1, f32), b1.reshape(1, -1).astype(f32),
            np.ascontiguousarray(W2, f32), b2.reshape(1, -1).astype(f32),
            np.ascontiguousarray(W3, f32), b3.reshape(1, -1).astype(f32),
        )

    W1l, b1l, W2l, b2l, W3l, b3l = fold(dec_local)
    W1r, b1r, W2r, b2r, W3r, b3r = fold(dec_remote)

    shared = {
        "nfT": nfT, "Wg": np.ascontiguousarray(W), "Wsd": Wsd,
        "bgat": gat["bias"].reshape(1, -1).astype(f32),
        "W1l": W1l, "b1l": b1l, "W2l": W2l, "b2l": b2l, "W3l": W3l, "b3l": b3l,
        "W1r": W1r, "b1r": b1r, "W2r": W2r, "b2r": b2r, "W3r": W3r, "b3r": b3r,
    }
    in_maps = []
    for c in range(NCORES):
        sl = slice(c * SH, (c + 1) * SH)
        m = dict(shared)
        m["nfT_loc"] = np.ascontiguousarray(nfT[:, sl])
        m["nfT_rem"] = np.ascontiguousarray(nfT[:, L + c * SH : L + (c + 1) * SH])
        in_maps.append(m)

    nc = _get_nc()
    res = run_bass_kernel_spmd(nc, in_maps, core_ids=list(range(NCORES)), trace=trace)
    _NC_CACHE["last_results"] = res
    outs = res.results

    # ---- host unshard / assembly ----
    dense = np.concatenate(
        [outs[c]["alpha_d"].reshape(SH, HEADS, L) for c in range(NCORES)], axis=0
    )  # [d, h, s]
    a_rl = np.concatenate([outs[c]["asmall"][:, 0:4] for c in range(NCORES)], axis=0)
    a_lr = np.concatenate([outs[c]["asmall"][:, 4:8] for c in range(NCORES)], axis=0)
    a_rr = np.concatenate([outs[c]["asmall"][:, 8:12] for c in range(NCORES)], axis=0)

    # clique edges in (s, d) order, diag removed
    per_edge = np.ascontiguousarray(np.transpose(dense, (2, 0, 1))).reshape(
        L * L, HEADS
    )
    mask = ~np.eye(L, dtype=bool).reshape(-1)
    clique = per_edge[mask]
    idx = np.arange(L)
    self_local = dense[idx, :, idx]  # [L, H]
    alpha = np.concatenate(
        [clique, a_lr, a_rl, self_local, a_rr], axis=0
    ).astype(np.float32)

    rec_local = np.empty((N, IN_DIM), np.float32)
    rec_remote = np.empty((N, IN_DIM), np.float32)
    for c in range(NCORES):
        sl = slice(c * SH, (c + 1) * SH)
        slr = slice(L + c * SH, L + (c + 1) * SH)
        rec_local[sl] = outs[c]["rec_l"][0:SH]
        rec_local[slr] = outs[c]["rec_l"][SH : 2 * SH]
        rec_remote[sl] = outs[c]["rec_r"][0:SH]
        rec_remote[slr] = outs[c]["rec_r"][SH : 2 * SH]
    reconstructed = np.where((node_types_np == 1)[:, None], rec_local, rec_remote)

    return reconstructed.astype(np.float32), edge_index_np, alpha


# revision 8
# speedup vs baseline: 1.5134x; 1.2944x over previous
"""Self-contained Trainium2 (Bass/Tile) kernel for the DeviceGAT problem.

Computes, on 8 NeuronCores, the GAT layer + LayerNorm + two decoder MLPs of
reference.py, exploiting the deterministic graph structure:
  - edges = dense local clique (1024 local nodes, no self edges)
            + local->remote pairs (i -> L+i)
            + remote->local pairs (L+i -> i)
            + self loops for all 2048 nodes
  - so each local dst d has in-edges from all 1024 local nodes (incl. itself
    via the self loop) plus remote node L+d; each remote dst L+i has in-edges
    {i, L+i}.

Sharding: destination rows are sharded 8 ways (128 local dst rows + the
matching 128 remote dst rows per core).  Node features / params replicated.

Scores are built directly in transposed [src, dst] layout (a_dst broadcast by
a tiny selector matmul, a_src as the per-partition activation bias), so the
attention needs NO on-chip transposes.  exp(scores) is written to HBM
unnormalized together with the per-(dst, head) softmax reciprocal R; the host
applies R while permuting the dense block into the reference per-edge order.
The aggregation matmul contracts src on the partition axis (exb^T @ [x | 1])
in bf16, producing both the weighted message sum and the softmax denominator
in one PSUM accumulation chain.
"""

import os
import sys
import types
import numpy as np

# ---------------------------------------------------------------------------
# Problem constants (from the reference problem definition; deterministic).
L = 1024
N = 2 * L
MAPPED = 32
HID = 64
HEADS = 4
OUT = HID * HEADS        # 256
IN_DIM = 128
NEG_SLOPE = 0.2
EPS = 1e-5
NCORES = 8
SH = L // NCORES         # 128 dst rows per core
NLOC_T = L // 128        # 8 local node tiles


def _ensure_axon_hooks_stub():
    """run_bass_kernel_spmd(trace=True) imports antenv.axon_hooks; provide a
    graceful stub when the image lacks it so tracing degrades instead of
    crashing.  (Harness runs trace=False and never hits this, but be safe.)"""
    try:
        import antenv.axon_hooks  # noqa: F401
        return
    except Exception:
        pass
    try:
        import antenv
    except Exception:
        antenv = types.ModuleType("antenv")
        sys.modules["antenv"] = antenv
    mod = types.ModuleType("antenv.axon_hooks")
    mod._HOOK = None

    def set_axon_ntff_profile_hook(hook):
        mod._HOOK = hook

    def get_axon_ntff_profile_hook():
        if mod._HOOK is not None:
            return mod._HOOK
        so = "/opt/axon/libaxon_pjrt.so"
        if os.path.exists(so):
            import contextlib
            import ctypes

            try:
                lib = ctypes.CDLL(so)
            except OSError:
                return None
            if not hasattr(lib, "axon_start_nrt_profile"):
                return None
            lib.axon_start_nrt_profile.argtypes = [
                ctypes.POINTER(ctypes.c_int64),
                ctypes.c_size_t,
            ]
            lib.axon_start_nrt_profile.restype = ctypes.c_int64
            lib.axon_stop_nrt_profile.argtypes = [ctypes.c_char_p]
            lib.axon_stop_nrt_profile.restype = ctypes.c_int64

            @contextlib.contextmanager
            def _hook(output_dir, device_ids):
                import jax

                jax.devices()
                if device_ids:
                    ids = (ctypes.c_int64 * len(device_ids))(*device_ids)
                    rc = lib.axon_start_nrt_profile(ids, len(device_ids))
                else:
                    rc = lib.axon_start_nrt_profile(None, 0)
                if rc != 0:
                    raise RuntimeError(f"axon_start_nrt_profile rc={rc}")
                try:
                    yield
                finally:
                    n = lib.axon_stop_nrt_profile(str(output_dir).encode())
                    print(f"profile: {n} file(s) in {output_dir}", file=sys.stderr)

            mod._HOOK = _hook
            return mod._HOOK
        return None

    mod.set_axon_ntff_profile_hook = set_axon_ntff_profile_hook
    mod.get_axon_ntff_profile_hook = get_axon_ntff_profile_hook
    sys.modules["antenv.axon_hooks"] = mod


# ---------------------------------------------------------------------------
# Bass kernel builder


def _build_nc():
    from contextlib import ExitStack

    import concourse.tile as tile
    from concourse import bacc, mybir
    from concourse.masks import make_identity

    f32 = mybir.dt.float32
    bf16 = mybir.dt.bfloat16
    AF = mybir.ActivationFunctionType
    WEXT = OUT + 2 * HEADS  # 264: [Wg | Wsrc | Wdst]

    nc = bacc.Bacc(
        trn_type="TRN2", target_bir_lowering=False, debug=False, num_devices=NCORES
    )

    # ---- I/O ----
    nfT = nc.dram_tensor("nfT", [MAPPED, N], f32, kind="ExternalInput")
    nfT_loc = nc.dram_tensor("nfT_loc", [MAPPED, SH], f32, kind="ExternalInput")
    nfT_rem = nc.dram_tensor("nfT_rem", [MAPPED, SH], f32, kind="ExternalInput")
    Wg = nc.dram_tensor("Wg", [MAPPED, WEXT], f32, kind="ExternalInput")
    bgat = nc.dram_tensor("bgat", [1, OUT], f32, kind="ExternalInput")
    decs = {}
    for tag in ("l", "r"):
        decs[tag] = {
            "W1": nc.dram_tensor(f"W1{tag}", [OUT, 128], f32, kind="ExternalInput"),
            "b1": nc.dram_tensor(f"b1{tag}", [1, 128], f32, kind="ExternalInput"),
            "W2": nc.dram_tensor(f"W2{tag}", [128, 64], f32, kind="ExternalInput"),
            "b2": nc.dram_tensor(f"b2{tag}", [1, 64], f32, kind="ExternalInput"),
            "W3": nc.dram_tensor(f"W3{tag}", [64, 128], f32, kind="ExternalInput"),
            "b3": nc.dram_tensor(f"b3{tag}", [1, 128], f32, kind="ExternalInput"),
        }

    # dense unnormalized exp(scores), transposed layout, raw per-head blocks
    # [p, t, d]: src node s = t*128+p, dst column d
    alpha_t = nc.dram_tensor("alpha_t", [128, HEADS * L], f32, kind="ExternalOutput")
    # a_rl | a_lr | a_rr | R  (R = per-(d,h) softmax reciprocal)
    asmall = nc.dram_tensor("asmall", [SH, 16], f32, kind="ExternalOutput")
    rec_l = nc.dram_tensor("rec_l", [2 * SH, IN_DIM], f32, kind="ExternalOutput")
    rec_r = nc.dram_tensor("rec_r", [2 * SH, IN_DIM], f32, kind="ExternalOutput")

    with tile.TileContext(nc) as tc, ExitStack() as ctx:
        consts = ctx.enter_context(tc.tile_pool(name="consts", bufs=1))
        big = ctx.enter_context(tc.tile_pool(name="big", bufs=2))
        persist = ctx.enter_context(tc.tile_pool(name="persist", bufs=1))
        small = ctx.enter_context(tc.tile_pool(name="small", bufs=2))
        psB = ctx.enter_context(tc.tile_pool(name="psB", bufs=2, space="PSUM"))
        psG = ctx.enter_context(tc.tile_pool(name="psG", bufs=2, space="PSUM"))
        psT = ctx.enter_context(tc.tile_pool(name="psT", bufs=2, space="PSUM"))
        psX = ctx.enter_context(tc.tile_pool(name="psX", bufs=2, space="PSUM"))

        # ---- load constants ----
        sb_nfT = consts.tile([MAPPED, N], f32)
        nc.sync.dma_start(sb_nfT[:], nfT[:])
        sb_nfT_loc = consts.tile([MAPPED, SH], f32)
        nc.sync.dma_start(sb_nfT_loc[:], nfT_loc[:])
        sb_nfT_rem = consts.tile([MAPPED, SH], f32)
        nc.sync.dma_start(sb_nfT_rem[:], nfT_rem[:])
        sb_Wg = consts.tile([MAPPED, WEXT], f32)
        nc.sync.dma_start(sb_Wg[:], Wg[:])
        sb_bgat = consts.tile([1, OUT], f32)
        nc.sync.dma_start(sb_bgat[:], bgat[:])
        sbd = {}
        for tag in ("l", "r"):
            d = decs[tag]
            sbd[tag] = {
                "W1": consts.tile([128, 2, 128], f32, tag=f"W1{tag}", name=f"sbW1{tag}"),
                "b1": consts.tile([1, 128], f32, tag=f"b1{tag}", name=f"sbb1{tag}"),
                "W2": consts.tile([128, 64], f32, tag=f"W2{tag}", name=f"sbW2{tag}"),
                "b2": consts.tile([1, 64], f32, tag=f"b2{tag}", name=f"sbb2{tag}"),
                "W3": consts.tile([64, 128], f32, tag=f"W3{tag}", name=f"sbW3{tag}"),
                "b3": consts.tile([1, 128], f32, tag=f"b3{tag}", name=f"sbb3{tag}"),
            }
            nc.sync.dma_start(
                sbd[tag]["W1"][:], d["W1"].rearrange("(a p) n -> p a n", p=128)
            )
            for k in ("b1", "W2", "b2", "W3", "b3"):
                nc.sync.dma_start(sbd[tag][k][:], d[k][:])

        eps_col = consts.tile([128, 1], f32)
        nc.vector.memset(eps_col[:], EPS)
        ones_row = consts.tile([1, 128], f32)
        nc.vector.memset(ones_row[:], 1.0)
        ident = consts.tile([128, 128], f32)
        make_identity(nc, ident[:])
        # head-selector: sel4[k, h*128+m] = (k == h); used as k=4 lhsT to
        # broadcast row h of a [4, x] tile across 128 partitions
        sel4 = consts.tile([HEADS, HEADS * 128], f32)
        nc.gpsimd.memset(sel4[:], 0.0)
        sel4v = sel4.rearrange("p (a b) -> p a b", a=HEADS)
        nc.gpsimd.affine_select(
            out=sel4v,
            in_=sel4v,
            compare_op=mybir.AluOpType.not_equal,
            fill=1.0,
            base=0,
            pattern=[[-1, HEADS], [0, 128]],
            channel_multiplier=1,
        )

        # ---- projections on PE: x|a_src|a_dst per node tile ----
        x_bf = []   # bf16 [128, 4, 65] tiles: per head [x_h | ones]
        A_t = []    # f32 [128, 8]: a_src cols 0:4, a_dst cols 4:8
        for t in range(NLOC_T):
            ps = psX.tile([128, WEXT], f32, tag="psx")
            nc.tensor.matmul(ps[:], sb_nfT[:, t * 128 : (t + 1) * 128], sb_Wg[:])
            xb = persist.tile([128, HEADS, HID + 1], bf16, tag=f"xbf{t}", name=f"xbf{t}")
            nc.vector.memset(xb[:], 1.0)
            nc.vector.tensor_copy(
                xb[:, :, 0:HID], ps[:, 0:OUT].rearrange("p (h c) -> p h c", h=HEADS)
            )
            at = persist.tile([128, 2 * HEADS], f32, tag=f"at{t}", name=f"at{t}")
            nc.scalar.copy(at[:], ps[:, OUT:WEXT])
            x_bf.append(xb)
            A_t.append(at)

        ps = psX.tile([128, WEXT], f32, tag="psx")
        nc.tensor.matmul(ps[:], sb_nfT_rem[:], sb_Wg[:])
        x_rem = persist.tile([128, OUT], f32, tag="xrem")
        nc.scalar.copy(x_rem[:], ps[:, 0:OUT])
        A_rem = persist.tile([128, 2 * HEADS], f32, tag="arem")
        nc.vector.tensor_copy(A_rem[:], ps[:, OUT:WEXT])

        ps = psX.tile([128, WEXT], f32, tag="psx")
        nc.tensor.matmul(ps[:], sb_nfT_loc[:], sb_Wg[:])
        x_shard = persist.tile([128, OUT], f32, tag="xshard")
        nc.vector.tensor_copy(x_shard[:], ps[:, 0:OUT])
        A_loc = persist.tile([128, 2 * HEADS], f32, tag="aloc")
        nc.scalar.copy(A_loc[:], ps[:, OUT:WEXT])

        # a_dst of the shard as rows: [4, 128]
        psd = psX.tile([128, WEXT], f32, tag="psx")
        nc.tensor.matmul(
            psd[0:HEADS, 0:SH], sb_Wg[:, OUT + HEADS : WEXT], sb_nfT_loc[:]
        )
        a_dstT = consts.tile([HEADS, SH], f32)
        nc.scalar.copy(a_dstT[:], psd[0:HEADS, 0:SH])

        # Bgat broadcast tile (for h_rem / h_loc bias)
        psb = psX.tile([128, WEXT], f32, tag="psx")
        nc.tensor.matmul(psb[:, 0:OUT], ones_row[:], sb_bgat[:])
        sb_Bg = persist.tile([128, OUT], f32, tag="bg")
        nc.scalar.copy(sb_Bg[:], psb[:, 0:OUT])

        # ---- special-edge scores: e_rl | e_lr | e_rr  [128, 12] ----
        E3 = persist.tile([128, 12], f32, tag="E3")
        nc.vector.tensor_add(E3[:, 0:4], A_rem[:, 0:4], A_loc[:, 4:8])
        nc.vector.tensor_add(E3[:, 4:8], A_loc[:, 0:4], A_rem[:, 4:8])
        nc.vector.tensor_add(E3[:, 8:12], A_rem[:, 0:4], A_rem[:, 4:8])
        LR3 = persist.tile([128, 12], f32, tag="LR3")
        nc.scalar.activation(LR3[:], E3[:], AF.Prelu, alpha=NEG_SLOPE)
        EXP3 = persist.tile([128, 12], f32, tag="EXP3")
        nc.scalar.activation(EXP3[:], LR3[:], AF.Exp)

        # remote-dst softmax (2 edges) and h_rem
        asm = persist.tile([128, 16], f32, tag="asm")  # a_rl | a_lr | a_rr | R
        Zr = persist.tile([128, 4], f32, tag="Zr")
        nc.vector.tensor_add(Zr[:], EXP3[:, 4:8], EXP3[:, 8:12])
        Rr = persist.tile([128, 4], f32, tag="Rr")
        nc.vector.reciprocal(Rr[:], Zr[:])
        nc.vector.tensor_mul(asm[:, 4:8], EXP3[:, 4:8], Rr[:])
        nc.vector.tensor_mul(asm[:, 8:12], EXP3[:, 8:12], Rr[:])

        hrem_t1 = persist.tile([128, OUT], f32, tag="hrem_t1")
        hrem_t2 = persist.tile([128, OUT], f32, tag="hrem_t2")
        for h in range(HEADS):
            hs = slice(h * HID, (h + 1) * HID)
            nc.vector.tensor_scalar_mul(hrem_t1[:, hs], x_shard[:, hs], asm[:, 4 + h : 5 + h])
            nc.vector.tensor_scalar_mul(hrem_t2[:, hs], x_rem[:, hs], asm[:, 8 + h : 9 + h])
        h_rem = persist.tile([128, OUT], f32, tag="h_rem")
        nc.vector.tensor_add(h_rem[:], hrem_t1[:], hrem_t2[:])
        nc.vector.tensor_add(h_rem[:], h_rem[:], sb_Bg[:])

        # ---- dense attention per head (transposed [s, d] layout) ----
        h_loc = persist.tile([128, OUT], f32, tag="h_loc")
        for h in range(HEADS):
            # Bh[s_p, d] = a_dst[d]  (selector matmul broadcast)
            psBh = psB.tile([128, SH], f32, tag="Bh")
            nc.tensor.matmul(psBh[:], sel4[:, h * 128 : (h + 1) * 128], a_dstT[:])
            # e_T[p, t, d] = prelu(a_src[t*128+p] + a_dst[d])
            e_T = big.tile([128, NLOC_T, SH], f32, tag="eT")
            for t in range(NLOC_T):
                nc.scalar.activation(
                    e_T[:, t, :], psBh[:], AF.Prelu,
                    bias=A_t[t][:, h : h + 1], alpha=NEG_SLOPE,
                )
            ex_T = big.tile([128, NLOC_T, SH], f32, tag="exT")
            nc.scalar.activation(
                ex_T.rearrange("p a b -> p (a b)"),
                e_T.rearrange("p a b -> p (a b)"),
                AF.Exp,
            )
            # unnormalized exp block straight to HBM (host normalizes)
            nc.sync.dma_start(
                alpha_t[:, h * L : (h + 1) * L],
                ex_T.rearrange("p a b -> p (a b)"),
            )
            exb = big.tile([128, NLOC_T, SH], bf16, tag="exb")
            nc.vector.tensor_copy(exb[:], ex_T[:])

            # aggregation + Z in one accumulation: exb[s,:]^T @ [x_h | 1]
            psagg = psG.tile([128, HID + 1], f32, tag="agg")
            for t in range(NLOC_T):
                nc.tensor.matmul(
                    psagg[:],
                    exb[:, t, :],
                    x_bf[t][:, h, :],
                    start=(t == 0),
                    stop=(t == NLOC_T - 1),
                )
            # Z = col HID + exp(e_rl);  R = 1/Z
            zf = small.tile([128, 1], f32, tag="zf")
            nc.vector.tensor_add(zf[:], psagg[:, HID : HID + 1], EXP3[:, h : h + 1])
            nc.vector.reciprocal(asm[:, 12 + h : 13 + h], zf[:])
            nc.vector.tensor_mul(
                asm[:, h : h + 1], EXP3[:, h : h + 1], asm[:, 12 + h : 13 + h]
            )
            # h_loc slice = (agg + exp(e_rl) * x_rem) * R
            t1 = small.tile([128, HID], f32, tag="t1")
            nc.vector.tensor_scalar_mul(
                t1[:], x_rem[:, h * HID : (h + 1) * HID], EXP3[:, h : h + 1]
            )
            t2 = small.tile([128, HID], f32, tag="t2")
            nc.vector.tensor_add(t2[:], psagg[:, 0:HID], t1[:])
            nc.vector.tensor_scalar_mul(
                h_loc[:, h * HID : (h + 1) * HID], t2[:], asm[:, 12 + h : 13 + h]
            )

        nc.vector.tensor_add(h_loc[:], h_loc[:], sb_Bg[:])
        nc.sync.dma_start(asmall[:], asm[:])

        # ---- layernorm helper ----
        BNS = nc.vector.BN_STATS_DIM
        BNA = nc.vector.BN_AGGR_DIM

        def layer_norm(x_t, width, tagp):
            st = small.tile([128, BNS], f32, tag="bnst")
            nc.vector.bn_stats(st[:], x_t[:, 0:width])
            mv = small.tile([128, BNA], f32, tag="bnmv")
            nc.vector.bn_aggr(mv[:], st[:])
            sdev = small.tile([128, 1], f32, tag="sdev")
            nc.scalar.activation(sdev[:], mv[:, 1:2], AF.Sqrt, bias=eps_col[:])
            rstd = small.tile([128, 1], f32, tag="rstd")
            nc.vector.reciprocal(rstd[:], sdev[:])
            negmr = small.tile([128, 1], f32, tag="negmr")
            nc.vector.tensor_scalar(
                negmr[:], mv[:, 0:1], rstd[:], -1.0,
                op0=mybir.AluOpType.mult, op1=mybir.AluOpType.mult,
            )
            out_t = big.tile([128, width], f32, tag=tagp)
            nc.scalar.activation(
                out_t[:], x_t[:, 0:width], AF.Identity, bias=negmr[:], scale=rstd[:]
            )
            return out_t

        hn_loc = layer_norm(h_loc, OUT, "hn")
        hn_rem = layer_norm(h_rem, OUT, "hn")

        # ---- transpose normalized h for decoder matmuls ----
        hTa = persist.tile([128, 2, 128], f32, tag="hTa")
        hTb = persist.tile([128, 2, 128], f32, tag="hTb")
        for i, (src_t, nt) in enumerate(((hn_loc, 0), (hn_rem, 1))):
            psTt = psT.tile([128, 4, 128], f32, tag="psT")
            nc.tensor.transpose(psTt[:, 0, :], src_t[:, 0:128], ident[:])
            nc.tensor.transpose(psTt[:, 1, :], src_t[:, 128:256], ident[:])
            if i == 0:
                nc.scalar.copy(hTa[:, nt, :], psTt[:, 0, :])
                nc.vector.tensor_copy(hTb[:, nt, :], psTt[:, 1, :])
            else:
                nc.vector.tensor_copy(hTa[:, nt, :], psTt[:, 0, :])
                nc.scalar.copy(hTb[:, nt, :], psTt[:, 1, :])

        # ---- decoders ----
        rec_sb = {}
        for di, tag in enumerate(("l", "r")):
            p = sbd[tag]
            rec_t = persist.tile([128, 2, IN_DIM], f32, tag=f"rec{tag}", name=f"rec{tag}")
            rec_sb[tag] = rec_t
            for nt in range(2):
                ps1 = psX.tile([128, WEXT], f32, tag="psx")
                nc.tensor.matmul(
                    ps1[:, 0:128], ones_row[:], p["b1"][:], start=True, stop=False
                )
                nc.tensor.matmul(
                    ps1[:, 0:128], hTa[:, nt, :], p["W1"][:, 0, :],
                    start=False, stop=False,
                )
                nc.tensor.matmul(
                    ps1[:, 0:128], hTb[:, nt, :], p["W1"][:, 1, :],
                    start=False, stop=True,
                )
                r1 = big.tile([128, 128], f32, tag="r1")
                nc.scalar.activation(r1[:], ps1[:, 0:128], AF.Relu)
                n1 = layer_norm(r1, 128, "n1")
                psn = psT.tile([128, 4, 128], f32, tag="psT")
                nc.tensor.transpose(psn[:, 0, :], n1[:], ident[:])
                n1T = big.tile([128, 128], f32, tag="n1T")
                if nt == 0:
                    nc.scalar.copy(n1T[:], psn[:, 0, :])
                else:
                    nc.vector.tensor_copy(n1T[:], psn[:, 0, :])

                ps2 = psX.tile([128, WEXT], f32, tag="psx")
                nc.tensor.matmul(
                    ps2[:, 0:64], ones_row[:], p["b2"][:], start=True, stop=False
                )
                nc.tensor.matmul(
                    ps2[:, 0:64], n1T[:], p["W2"][:], start=False, stop=True
                )
                r2 = big.tile([128, 64], f32, tag="r2")
                nc.scalar.activation(r2[:], ps2[:, 0:64], AF.Relu)
                n2 = layer_norm(r2, 64, "n2")
                psn2 = psT.tile([128, 4, 128], f32, tag="psT")
                nc.tensor.transpose(psn2[0:64, 0, :], n2[:, 0:64], ident[:])
                n2T = big.tile([64, 128], f32, tag="n2T")
                if nt == 0:
                    nc.vector.tensor_copy(n2T[:], psn2[0:64, 0, :])
                else:
                    nc.scalar.copy(n2T[:], psn2[0:64, 0, :])

                ps3 = psX.tile([128, WEXT], f32, tag="psx")
                nc.tensor.matmul(
                    ps3[:, 0:IN_DIM], ones_row[:], p["b3"][:], start=True, stop=False
                )
                nc.tensor.matmul(
                    ps3[:, 0:IN_DIM], n2T[:], p["W3"][:], start=False, stop=True
                )
                if (nt + di) % 2 == 0:
                    nc.scalar.copy(rec_t[:, nt, :], ps3[:, 0:IN_DIM])
                else:
                    nc.vector.tensor_copy(rec_t[:, nt, :], ps3[:, 0:IN_DIM])

        nc.sync.dma_start(rec_l.rearrange("(a p) n -> p a n", p=128), rec_sb["l"][:])
        nc.sync.dma_start(rec_r.rearrange("(a p) n -> p a n", p=128), rec_sb["r"][:])

    nc.compile()
    return nc


_NC_CACHE = {}


def _get_nc():
    if "nc" not in _NC_CACHE:
        _NC_CACHE["nc"] = _build_nc()
    return _NC_CACHE["nc"]


# ---------------------------------------------------------------------------
# Host side


def _expected_edge_index():
    local = np.arange(L)
    s = np.repeat(local, L)
    d = np.tile(local, L)
    m = s != d
    src = np.concatenate([s[m], local, L + local, np.arange(N)])
    dst = np.concatenate([d[m], L + local, local, np.arange(N)])
    return np.stack([src, dst])


def _np_reference_fallback(node_features, gat, norm, dec_local, dec_remote,
                           node_types, edge_index):
    """Pure-numpy replica of the reference; used only if the edge structure
    is not the expected deterministic pattern."""
    x = node_features @ gat["W"]
    xh = x.reshape(N, HEADS, HID)
    a_src = (xh * gat["att_src"]).sum(-1)
    a_dst = (xh * gat["att_dst"]).sum(-1)
    src, dst = edge_index[0], edge_index[1]
    e = a_src[src] + a_dst[dst]
    e = np.where(e >= 0, e, NEG_SLOPE * e).astype(np.float32)
    emax = np.full((N, HEADS), -np.inf, np.float32)
    np.maximum.at(emax, dst, e)
    ex = np.exp(e - emax[dst])
    zs = np.zeros((N, HEADS), np.float32)
    np.add.at(zs, dst, ex)
    alpha = ex / zs[dst]
    msg = xh[src] * alpha[:, :, None]
    out = np.zeros((N, HEADS, HID), np.float32)
    np.add.at(out, dst, msg)
    out = out.reshape(N, OUT) + gat["bias"]

    def ln(v, g, b):
        m = v.mean(-1, keepdims=True)
        var = ((v - m) ** 2).mean(-1, keepdims=True)
        return (v - m) / np.sqrt(var + EPS) * g + b

    h = ln(out, norm["g"], norm["b"])

    def dec(v, p):
        t = ln(np.maximum(v @ p["W1"] + p["b1"], 0), p["g1"], p["b1n"])
        t = ln(np.maximum(t @ p["W2"] + p["b2"], 0), p["g2"], p["b2n"])
        return t @ p["W3"] + p["b3"]

    rl = dec(h, dec_local)
    rr = dec(h, dec_remote)
    rec = np.where((node_types == 1)[:, None], rl, rr)
    return rec.astype(np.float32), edge_index, alpha.astype(np.float32)


def _to_np(v):
    return {k: _to_np(x) for k, x in v.items()} if isinstance(v, dict) else np.asarray(v)


def kernel(node_features, gat, norm, dec_local, dec_remote, node_types,
           edge_index, trace=False):
    _ensure_axon_hooks_stub()
    node_features = np.asarray(node_features, np.float32)
    gat, norm = _to_np(gat), _to_np(norm)
    dec_local, dec_remote = _to_np(dec_local), _to_np(dec_remote)
    node_types_np = np.asarray(node_types)
    edge_index_np = np.asarray(edge_index)

    if not np.array_equal(edge_index_np.astype(np.int64), _expected_edge_index()):
        return _np_reference_fallback(
            node_features, gat, norm, dec_local, dec_remote,
            node_types_np, edge_index_np,
        )

    from concourse.bass_utils import run_bass_kernel_spmd

    f32 = np.float32
    W = gat["W"].astype(f32)
    att_src = gat["att_src"].astype(f32)
    att_dst = gat["att_dst"].astype(f32)
    Wsrc = (W.reshape(MAPPED, HEADS, HID) * att_src[None]).sum(-1)
    Wdst = (W.reshape(MAPPED, HEADS, HID) * att_dst[None]).sum(-1)
    Wg_ext = np.ascontiguousarray(np.concatenate([W, Wsrc, Wdst], axis=1))
    nfT = np.ascontiguousarray(node_features.T)

    # fold LN affine params into the following linear layer
    def fold(dec):
        g, b = norm["g"].astype(f32), norm["b"].astype(f32)
        W1 = g[:, None] * dec["W1"]
        b1 = dec["b1"] + b @ dec["W1"]
        W2 = dec["g1"][:, None] * dec["W2"]
        b2 = dec["b2"] + dec["b1n"] @ dec["W2"]
        W3 = dec["g2"][:, None] * dec["W3"]
        b3 = dec["b3"] + dec["b2n"] @ dec["W3"]
        return (
            np.ascontiguousarray(W1, f32), b1.reshape(1, -1).astype(f32),
            np.ascontiguousarray(W2, f32), b2.reshape(1, -1).astype(f32),
            np.ascontiguousarray(W3, f32), b3.reshape(1, -1).astype(f32),
        )

    W1l, b1l, W2l, b2l, W3l, b3l = fold(dec_local)
    W1r, b1r, W2r, b2r, W3r, b3r = fold(dec_remote)

    shared = {
        "nfT": nfT, "Wg": Wg_ext,
        "bgat": gat["bias"].reshape(1, -1).astype(f32),
        "W1l": W1l, "b1l": b1l, "W2l": W2l, "b2l": b2l, "W3l": W3l, "b3l": b3l,
        "W1r": W1r, "b1r": b1r, "W2r": W2r, "b2r": b2r, "W3r": W3r, "b3r": b3r,
    }
    in_maps = []
    for c in range(NCORES):
        m = dict(shared)
        m["nfT_loc"] = np.ascontiguousarray(nfT[:, c * SH : (c + 1) * SH])
        m["nfT_rem"] = np.ascontiguousarray(nfT[:, L + c * SH : L + (c + 1) * SH])
        in_maps.append(m)

    nc = _get_nc()
    res = run_bass_kernel_spmd(nc, in_maps, core_ids=list(range(NCORES)), trace=trace)
    _NC_CACHE["last_results"] = res
    outs = res.results

    # ---- host unshard / assembly ----
    # alphaT_full[s, d, h] = exp(e) * R
    alphaT_full = np.empty((L, L, HEADS), np.float32)
    a_rl = np.empty((L, HEADS), np.float32)
    a_lr = np.empty((L, HEADS), np.float32)
    a_rr = np.empty((L, HEADS), np.float32)
    for c in range(NCORES):
        sl = slice(c * SH, (c + 1) * SH)
        sm = outs[c]["asmall"]
        a_rl[sl], a_lr[sl], a_rr[sl] = sm[:, 0:4], sm[:, 4:8], sm[:, 8:12]
        R = sm[:, 12:16]  # [d, h]
        blk = outs[c]["alpha_t"].reshape(128, HEADS, NLOC_T, SH)  # [p, h, t, d]
        # -> [s=t*128+p, d, h]
        blk = np.transpose(blk, (2, 0, 3, 1)).reshape(L, SH, HEADS)
        alphaT_full[:, sl, :] = blk * R[None, :, :]

    per_edge = alphaT_full.reshape(L * L, HEADS)
    mask = ~np.eye(L, dtype=bool).reshape(-1)
    clique = per_edge[mask]
    idx = np.arange(L)
    self_local = alphaT_full[idx, idx, :]
    alpha = np.concatenate(
        [clique, a_lr, a_rl, self_local, a_rr], axis=0
    ).astype(np.float32)

    rec_local = np.empty((N, IN_DIM), np.float32)
    rec_remote = np.empty((N, IN_DIM), np.float32)
    for c in range(NCORES):
        sl = slice(c * SH, (c + 1) * SH)
        slr = slice(L + c * SH, L + (c + 1) * SH)
        rec_local[sl] = outs[c]["rec_l"][0:SH]
        rec_local[slr] = outs[c]["rec_l"][SH : 2 * SH]
        rec_remote[sl] = outs[c]["rec_r"][0:SH]
        rec_remote[slr] = outs[c]["rec_r"][SH : 2 * SH]
    reconstructed = np.where((node_types_np == 1)[:, None], rec_local, rec_remote)

    return reconstructed.astype(np.float32), edge_index_np, alpha


# revision 12
# speedup vs baseline: 1.5485x; 1.0232x over previous
"""Self-contained Trainium2 (Bass/Tile) kernel for the DeviceGAT problem.

Computes, on 8 NeuronCores, the GAT layer + LayerNorm + two decoder MLPs of
reference.py, exploiting the deterministic graph structure:
  - edges = dense local clique (1024 local nodes, no self edges)
            + local->remote pairs (i -> L+i)
            + remote->local pairs (L+i -> i)
            + self loops for all 2048 nodes
  - so each local dst d has in-edges from all 1024 local nodes (incl. itself
    via the self loop) plus remote node L+d; each remote dst L+i has in-edges
    {i, L+i}.

Sharding: destination rows are sharded 8 ways (128 local dst rows + the
matching 128 remote dst rows per core).  Node features / params replicated.

Scores are built directly in transposed [src, dst] layout (a_dst broadcast by
a tiny selector matmul, a_src as the per-partition activation bias), so the
attention needs NO on-chip transposes.  exp(scores) is written to HBM
unnormalized together with the per-(dst, head) softmax reciprocal R; the host
applies R while permuting the dense block into the reference per-edge order.
The aggregation matmul contracts src on the partition axis (exb^T @ [x | 1])
in bf16, producing both the weighted message sum and the softmax denominator
in one PSUM accumulation chain.
"""

import os
import sys
import types
import numpy as np

# ---------------------------------------------------------------------------
# Problem constants (from the reference problem definition; deterministic).
L = 1024
N = 2 * L
MAPPED = 32
HID = 64
HEADS = 4
OUT = HID * HEADS        # 256
IN_DIM = 128
NEG_SLOPE = 0.2
EPS = 1e-5
NCORES = 8
SH = L // NCORES         # 128 dst rows per core
NLOC_T = L // 128        # 8 local node tiles


def _ensure_axon_hooks_stub():
    """run_bass_kernel_spmd(trace=True) imports antenv.axon_hooks; provide a
    graceful stub when the image lacks it so tracing degrades instead of
    crashing.  (Harness runs trace=False and never hits this, but be safe.)"""
    try:
        import antenv.axon_hooks  # noqa: F401
        return
    except Exception:
        pass
    try:
        import antenv
    except Exception:
        antenv = types.ModuleType("antenv")
        sys.modules["antenv"] = antenv
    mod = types.ModuleType("antenv.axon_hooks")
    mod._HOOK = None

    def set_axon_ntff_profile_hook(hook):
        mod._HOOK = hook

    def get_axon_ntff_profile_hook():
        if mod._HOOK is not None:
            return mod._HOOK
        so = "/opt/axon/libaxon_pjrt.so"
        if os.path.exists(so):
            import contextlib
            import ctypes

            try:
                lib = ctypes.CDLL(so)
            except OSError:
                return None
            if not hasattr(lib, "axon_start_nrt_profile"):
                return None
            lib.axon_start_nrt_profile.argtypes = [
                ctypes.POINTER(ctypes.c_int64),
                ctypes.c_size_t,
            ]
            lib.axon_start_nrt_profile.restype = ctypes.c_int64
            lib.axon_stop_nrt_profile.argtypes = [ctypes.c_char_p]
            lib.axon_stop_nrt_profile.restype = ctypes.c_int64

            @contextlib.contextmanager
            def _hook(output_dir, device_ids):
                import jax

                jax.devices()
                if device_ids:
                    ids = (ctypes.c_int64 * len(device_ids))(*device_ids)
                    rc = lib.axon_start_nrt_profile(ids, len(device_ids))
                else:
                    rc = lib.axon_start_nrt_profile(None, 0)
                if rc != 0:
                    raise RuntimeError(f"axon_start_nrt_profile rc={rc}")
                try:
                    yield
                finally:
                    n = lib.axon_stop_nrt_profile(str(output_dir).encode())
                    print(f"profile: {n} file(s) in {output_dir}", file=sys.stderr)

            mod._HOOK = _hook
            return mod._HOOK
        return None

    mod.set_axon_ntff_profile_hook = set_axon_ntff_profile_hook
    mod.get_axon_ntff_profile_hook = get_axon_ntff_profile_hook
    sys.modules["antenv.axon_hooks"] = mod


# ---------------------------------------------------------------------------
# Bass kernel builder


def _build_nc():
    from contextlib import ExitStack

    import concourse.tile as tile
    from concourse import bacc, mybir
    from concourse.masks import make_identity

    f32 = mybir.dt.float32
    bf16 = mybir.dt.bfloat16
    AF = mybir.ActivationFunctionType
    WEXT = OUT + 2 * HEADS  # 264: [Wg | Wsrc | Wdst]

    nc = bacc.Bacc(
        trn_type="TRN2", target_bir_lowering=False, debug=False, num_devices=NCORES
    )

    # ---- I/O ----
    nfT = nc.dram_tensor("nfT", [MAPPED, N], f32, kind="ExternalInput")
    nfT_loc = nc.dram_tensor("nfT_loc", [MAPPED, SH], f32, kind="ExternalInput")
    nfT_rem = nc.dram_tensor("nfT_rem", [MAPPED, SH], f32, kind="ExternalInput")
    Wg = nc.dram_tensor("Wg", [MAPPED, WEXT], f32, kind="ExternalInput")
    bgat = nc.dram_tensor("bgat", [1, OUT], f32, kind="ExternalInput")
    decs = {}
    for tag in ("l", "r"):
        decs[tag] = {
            "W1": nc.dram_tensor(f"W1{tag}", [OUT, 128], bf16, kind="ExternalInput"),
            "b1": nc.dram_tensor(f"b1{tag}", [128, 128], f32, kind="ExternalInput"),
            "W2": nc.dram_tensor(f"W2{tag}", [128, 64], bf16, kind="ExternalInput"),
            "b2": nc.dram_tensor(f"b2{tag}", [128, 64], f32, kind="ExternalInput"),
            "W3": nc.dram_tensor(f"W3{tag}", [64, 128], bf16, kind="ExternalInput"),
            "b3": nc.dram_tensor(f"b3{tag}", [128, 128], f32, kind="ExternalInput"),
        }

    # dense unnormalized exp(scores), transposed layout, raw per-head blocks
    # [p, t, d]: src node s = t*128+p, dst column d
    alpha_t = nc.dram_tensor("alpha_t", [128, HEADS * L], f32, kind="ExternalOutput")
    # a_rl | a_lr | a_rr | R  (R = per-(d,h) softmax reciprocal)
    asmall = nc.dram_tensor("asmall", [SH, 16], f32, kind="ExternalOutput")
    rec_l = nc.dram_tensor("rec_l", [2 * SH, IN_DIM], f32, kind="ExternalOutput")
    rec_r = nc.dram_tensor("rec_r", [2 * SH, IN_DIM], f32, kind="ExternalOutput")

    with tile.TileContext(nc) as tc, ExitStack() as ctx:
        consts = ctx.enter_context(tc.tile_pool(name="consts", bufs=1))
        big = ctx.enter_context(tc.tile_pool(name="big", bufs=2))
        persist = ctx.enter_context(tc.tile_pool(name="persist", bufs=1))
        small = ctx.enter_context(tc.tile_pool(name="small", bufs=2))
        psB = ctx.enter_context(tc.tile_pool(name="psB", bufs=2, space="PSUM"))
        psG = ctx.enter_context(tc.tile_pool(name="psG", bufs=2, space="PSUM"))
        psT = ctx.enter_context(tc.tile_pool(name="psT", bufs=2, space="PSUM"))
        psX = ctx.enter_context(tc.tile_pool(name="psX", bufs=2, space="PSUM"))

        # ---- load constants ----
        sb_nfT = consts.tile([MAPPED, N], f32)
        nc.sync.dma_start(sb_nfT[:], nfT[:])
        sb_nfT_loc = consts.tile([MAPPED, SH], f32)
        nc.sync.dma_start(sb_nfT_loc[:], nfT_loc[:])
        sb_nfT_rem = consts.tile([MAPPED, SH], f32)
        nc.sync.dma_start(sb_nfT_rem[:], nfT_rem[:])
        sb_Wg = consts.tile([MAPPED, WEXT], f32)
        nc.sync.dma_start(sb_Wg[:], Wg[:])
        sb_bgat = consts.tile([1, OUT], f32)
        nc.sync.dma_start(sb_bgat[:], bgat[:])
        sbd = {}
        for tag in ("l", "r"):
            d = decs[tag]
            sbd[tag] = {
                "W1": consts.tile([128, 2, 128], bf16, tag=f"W1{tag}", name=f"sbW1{tag}"),
                "b1": consts.tile([128, 128], f32, tag=f"b1{tag}", name=f"sbb1{tag}"),
                "W2": consts.tile([128, 64], bf16, tag=f"W2{tag}", name=f"sbW2{tag}"),
                "b2": consts.tile([128, 64], f32, tag=f"b2{tag}", name=f"sbb2{tag}"),
                "W3": consts.tile([64, 128], bf16, tag=f"W3{tag}", name=f"sbW3{tag}"),
                "b3": consts.tile([128, 128], f32, tag=f"b3{tag}", name=f"sbb3{tag}"),
            }
            nc.sync.dma_start(
                sbd[tag]["W1"][:], d["W1"].rearrange("(a p) n -> p a n", p=128)
            )
            for k in ("b1", "W2", "b2", "W3", "b3"):
                nc.sync.dma_start(sbd[tag][k][:], d[k][:])

        eps_col = consts.tile([128, 1], f32)
        nc.vector.memset(eps_col[:], EPS)
        ones_row = consts.tile([1, 128], f32)
        nc.vector.memset(ones_row[:], 1.0)
        ident = consts.tile([128, 128], f32)
        make_identity(nc, ident[:])
        identb = consts.tile([128, 128], bf16)
        make_identity(nc, identb[:])
        # head-selector: sel4[k, h*128+m] = (k == h); used as k=4 lhsT to
        # broadcast row h of a [4, x] tile across 128 partitions
        sel4 = consts.tile([HEADS, HEADS * 128], f32)
        nc.gpsimd.memset(sel4[:], 0.0)
        sel4v = sel4.rearrange("p (a b) -> p a b", a=HEADS)
        nc.gpsimd.affine_select(
            out=sel4v,
            in_=sel4v,
            compare_op=mybir.AluOpType.not_equal,
            fill=1.0,
            base=0,
            pattern=[[-1, HEADS], [0, 128]],
            channel_multiplier=1,
        )

        # ---- projections on PE: x|a_src|a_dst per node tile ----
        x_bf = []   # bf16 [128, 4, 65] tiles: per head [x_h | ones]
        A_t = []    # f32 [128, 8]: a_src cols 0:4, a_dst cols 4:8
        for t in range(NLOC_T):
            ps = psX.tile([128, WEXT], f32, tag="psx")
            nc.tensor.matmul(ps[:], sb_nfT[:, t * 128 : (t + 1) * 128], sb_Wg[:])
            xb = persist.tile([128, HEADS, HID + 1], bf16, tag=f"xbf{t}", name=f"xbf{t}")
            nc.vector.memset(xb[:], 1.0)
            nc.vector.tensor_copy(
                xb[:, :, 0:HID], ps[:, 0:OUT].rearrange("p (h c) -> p h c", h=HEADS)
            )
            at = persist.tile([128, 2 * HEADS], f32, tag=f"at{t}", name=f"at{t}")
            nc.scalar.copy(at[:], ps[:, OUT:WEXT])
            x_bf.append(xb)
            A_t.append(at)

        ps = psX.tile([128, WEXT], f32, tag="psx")
        nc.tensor.matmul(ps[:], sb_nfT_rem[:], sb_Wg[:])
        x_rem = persist.tile([128, OUT], f32, tag="xrem")
        nc.scalar.copy(x_rem[:], ps[:, 0:OUT])
        A_rem = persist.tile([128, 2 * HEADS], f32, tag="arem")
        nc.vector.tensor_copy(A_rem[:], ps[:, OUT:WEXT])

        ps = psX.tile([128, WEXT], f32, tag="psx")
        nc.tensor.matmul(ps[:], sb_nfT_loc[:], sb_Wg[:])
        x_shard = persist.tile([128, OUT], f32, tag="xshard")
        nc.vector.tensor_copy(x_shard[:], ps[:, 0:OUT])
        A_loc = persist.tile([128, 2 * HEADS], f32, tag="aloc")
        nc.scalar.copy(A_loc[:], ps[:, OUT:WEXT])

        # a_dst of the shard as rows: [4, 128]
        psd = psX.tile([128, WEXT], f32, tag="psx")
        nc.tensor.matmul(
            psd[0:HEADS, 0:SH], sb_Wg[:, OUT + HEADS : WEXT], sb_nfT_loc[:]
        )
        a_dstT = consts.tile([HEADS, SH], f32)
        nc.scalar.copy(a_dstT[:], psd[0:HEADS, 0:SH])

        # Bgat broadcast tile (for h_rem / h_loc bias)
        psb = psX.tile([128, WEXT], f32, tag="psx")
        nc.tensor.matmul(psb[:, 0:OUT], ones_row[:], sb_bgat[:])
        sb_Bg = persist.tile([128, OUT], f32, tag="bg")
        nc.scalar.copy(sb_Bg[:], psb[:, 0:OUT])

        # ---- special-edge scores: e_rl | e_lr | e_rr  [128, 12] ----
        E3 = persist.tile([128, 12], f32, tag="E3")
        nc.vector.tensor_add(E3[:, 0:4], A_rem[:, 0:4], A_loc[:, 4:8])
        nc.vector.tensor_add(E3[:, 4:8], A_loc[:, 0:4], A_rem[:, 4:8])
        nc.vector.tensor_add(E3[:, 8:12], A_rem[:, 0:4], A_rem[:, 4:8])
        LR3 = persist.tile([128, 12], f32, tag="LR3")
        nc.scalar.activation(LR3[:], E3[:], AF.Prelu, alpha=NEG_SLOPE)
        EXP3 = persist.tile([128, 12], f32, tag="EXP3")
        nc.scalar.activation(EXP3[:], LR3[:], AF.Exp)

        # remote-dst softmax (2 edges) and h_rem
        asm = persist.tile([128, 16], f32, tag="asm")  # a_rl | a_lr | a_rr | R
        Zr = persist.tile([128, 4], f32, tag="Zr")
        nc.vector.tensor_add(Zr[:], EXP3[:, 4:8], EXP3[:, 8:12])
        Rr = persist.tile([128, 4], f32, tag="Rr")
        nc.vector.reciprocal(Rr[:], Zr[:])
        nc.vector.tensor_mul(asm[:, 4:8], EXP3[:, 4:8], Rr[:])
        nc.vector.tensor_mul(asm[:, 8:12], EXP3[:, 8:12], Rr[:])

        hrem_t1 = persist.tile([128, OUT], f32, tag="hrem_t1")
        hrem_t2 = persist.tile([128, OUT], f32, tag="hrem_t2")
        for h in range(HEADS):
            hs = slice(h * HID, (h + 1) * HID)
            nc.vector.tensor_scalar_mul(hrem_t1[:, hs], x_shard[:, hs], asm[:, 4 + h : 5 + h])
            nc.vector.tensor_scalar_mul(hrem_t2[:, hs], x_rem[:, hs], asm[:, 8 + h : 9 + h])
        h_rem = persist.tile([128, OUT], f32, tag="h_rem")
        nc.vector.tensor_add(h_rem[:], hrem_t1[:], hrem_t2[:])
        nc.vector.tensor_add(h_rem[:], h_rem[:], sb_Bg[:])


        # ---- layernorm helper ----
        BNS = nc.vector.BN_STATS_DIM
        BNA = nc.vector.BN_AGGR_DIM

        def layer_norm(x_t, width, tagp):
            st = small.tile([128, BNS], f32, tag="bnst")
            nc.vector.bn_stats(st[:], x_t[:, 0:width])
            mv = small.tile([128, BNA], f32, tag="bnmv")
            nc.vector.bn_aggr(mv[:], st[:])
            sdev = small.tile([128, 1], f32, tag="sdev")
            nc.scalar.activation(sdev[:], mv[:, 1:2], AF.Sqrt, bias=eps_col[:])
            rstd = small.tile([128, 1], f32, tag="rstd")
            nc.vector.reciprocal(rstd[:], sdev[:])
            negmr = small.tile([128, 1], f32, tag="negmr")
            nc.vector.tensor_scalar(
                negmr[:], mv[:, 0:1], rstd[:], -1.0,
                op0=mybir.AluOpType.mult, op1=mybir.AluOpType.mult,
            )
            out_t = big.tile([128, width], bf16, tag=tagp)
            nc.scalar.activation(
                out_t[:], x_t[:, 0:width], AF.Identity, bias=negmr[:], scale=rstd[:]
            )
            return out_t


        # ---- decoder machinery (bf16 matmuls, f32 psum/LN) ----
        rec_sb = {
            "l": persist.tile([128, 2, IN_DIM], f32, tag="recl", name="recl"),
            "r": persist.tile([128, 2, IN_DIM], f32, tag="recr", name="recr"),
        }

        def transpose_h(hn_t, nt):
            """transpose normalized h [128, 256] bf16 -> two [128, 128] tiles"""
            psTt = psT.tile([128, 4, 128], bf16, tag="psTb")
            nc.tensor.transpose(psTt[:, 0, :], hn_t[:, 0:128], identb[:])
            nc.tensor.transpose(psTt[:, 1, :], hn_t[:, 128:256], identb[:])
            ha = big.tile([128, 128], bf16, tag=f"hTa{nt}")
            hb = big.tile([128, 128], bf16, tag=f"hTb{nt}")
            if nt == 0:
                nc.scalar.copy(ha[:], psTt[:, 0, :])
                nc.vector.tensor_copy(hb[:], psTt[:, 1, :])
            else:
                nc.vector.tensor_copy(ha[:], psTt[:, 0, :])
                nc.scalar.copy(hb[:], psTt[:, 1, :])
            return ha, hb

        def run_decoders_nt(nt, ha, hb):
            for di, tag in enumerate(("l", "r")):
                p = sbd[tag]
                rec_t = rec_sb[tag]
                ps1 = psX.tile([128, WEXT], f32, tag="psx")
                nc.tensor.matmul(ps1[:, 0:128], ha[:], p["W1"][:, 0, :],
                                 start=True, stop=False)
                nc.tensor.matmul(ps1[:, 0:128], hb[:], p["W1"][:, 1, :],
                                 start=False, stop=True)
                s1 = big.tile([128, 128], f32, tag="s1")
                nc.vector.tensor_add(s1[:], ps1[:, 0:128], p["b1"][:])
                r1 = big.tile([128, 128], f32, tag="r1")
                nc.scalar.activation(r1[:], s1[:], AF.Relu)
                n1 = layer_norm(r1, 128, "n1")
                psn = psT.tile([128, 4, 128], bf16, tag="psTb")
                nc.tensor.transpose(psn[:, 0, :], n1[:], identb[:])
                n1T = big.tile([128, 128], bf16, tag="n1T")
                if (nt + di) % 2 == 0:
                    nc.scalar.copy(n1T[:], psn[:, 0, :])
                else:
                    nc.vector.tensor_copy(n1T[:], psn[:, 0, :])

                ps2 = psX.tile([128, WEXT], f32, tag="psx")
                nc.tensor.matmul(ps2[:, 0:64], n1T[:], p["W2"][:],
                                 start=True, stop=True)
                s2 = big.tile([128, 64], f32, tag="s2")
                nc.vector.tensor_add(s2[:], ps2[:, 0:64], p["b2"][:])
                r2 = big.tile([128, 64], f32, tag="r2")
                nc.scalar.activation(r2[:], s2[:], AF.Relu)
                n2 = layer_norm(r2, 64, "n2")
                psn2 = psT.tile([128, 4, 128], bf16, tag="psTb")
                nc.tensor.transpose(psn2[0:64, 0, :], n2[:, 0:64], identb[:])
                n2T = big.tile([64, 128], bf16, tag="n2T")
                if (nt + di) % 2 == 0:
                    nc.vector.tensor_copy(n2T[:], psn2[0:64, 0, :])
                else:
                    nc.scalar.copy(n2T[:], psn2[0:64, 0, :])

                ps3 = psX.tile([128, WEXT], f32, tag="psx")
                nc.tensor.matmul(ps3[:, 0:IN_DIM], n2T[:], p["W3"][:],
                                 start=True, stop=True)
                if (nt + di) % 2 == 0:
                    nc.vector.tensor_add(rec_t[:, nt, :], ps3[:, 0:IN_DIM], p["b3"][:])
                else:
                    nc.vector.tensor_add(rec_t[:, nt, :], ps3[:, 0:IN_DIM], p["b3"][:])

        # remote half of the decoders can run during the attention head loop
        hn_rem = layer_norm(h_rem, OUT, "hn")
        ha1, hb1 = transpose_h(hn_rem, 1)
        run_decoders_nt(1, ha1, hb1)

        # ---- dense attention per head (transposed [s, d] layout) ----
        h_loc = persist.tile([128, OUT], f32, tag="h_loc")
        for h in range(HEADS):
            # Bh[s_p, d] = a_dst[d]  (selector matmul broadcast)
            psBh = psB.tile([128, SH], f32, tag="Bh")
            nc.tensor.matmul(psBh[:], sel4[:, h * 128 : (h + 1) * 128], a_dstT[:])
            # e_T[p, t, d] = prelu(a_src[t*128+p] + a_dst[d])
            e_T = big.tile([128, NLOC_T, SH], f32, tag="eT")
            for t in range(NLOC_T):
                nc.scalar.activation(
                    e_T[:, t, :], psBh[:], AF.Prelu,
                    bias=A_t[t][:, h : h + 1], alpha=NEG_SLOPE,
                )
            ex_T = big.tile([128, NLOC_T, SH], f32, tag="exT")
            nc.scalar.activation(
                ex_T.rearrange("p a b -> p (a b)"),
                e_T.rearrange("p a b -> p (a b)"),
                AF.Exp,
            )
            # unnormalized exp block straight to HBM (host normalizes)
            nc.sync.dma_start(
                alpha_t[:, h * L : (h + 1) * L],
                ex_T.rearrange("p a b -> p (a b)"),
            )
            exb = big.tile([128, NLOC_T, SH], bf16, tag="exb")
            nc.vector.tensor_copy(exb[:], ex_T[:])

            # aggregation + Z in one accumulation: exb[s,:]^T @ [x_h | 1]
            psagg = psG.tile([128, HID + 1], f32, tag="agg")
            for t in range(NLOC_T):
                nc.tensor.matmul(
                    psagg[:],
                    exb[:, t, :],
                    x_bf[t][:, h, :],
                    start=(t == 0),
                    stop=(t == NLOC_T - 1),
                )
            # Z = col HID + exp(e_rl);  R = 1/Z
            zf = small.tile([128, 1], f32, tag="zf")
            nc.vector.tensor_add(zf[:], psagg[:, HID : HID + 1], EXP3[:, h : h + 1])
            nc.vector.reciprocal(asm[:, 12 + h : 13 + h], zf[:])
            nc.vector.tensor_mul(
                asm[:, h : h + 1], EXP3[:, h : h + 1], asm[:, 12 + h : 13 + h]
            )
            # h_loc slice = (agg + exp(e_rl) * x_rem) * R
            t1 = small.tile([128, HID], f32, tag="t1")
            nc.vector.tensor_scalar_mul(
                t1[:], x_rem[:, h * HID : (h + 1) * HID], EXP3[:, h : h + 1]
            )
            t2 = small.tile([128, HID], f32, tag="t2")
            nc.vector.tensor_add(t2[:], psagg[:, 0:HID], t1[:])
            nc.vector.tensor_scalar_mul(
                h_loc[:, h * HID : (h + 1) * HID], t2[:], asm[:, 12 + h : 13 + h]
            )

        nc.vector.tensor_add(h_loc[:], h_loc[:], sb_Bg[:])
        nc.sync.dma_start(asmall[:], asm[:])

        hn_loc = layer_norm(h_loc, OUT, "hn")
        ha0, hb0 = transpose_h(hn_loc, 0)
        run_decoders_nt(0, ha0, hb0)

        nc.sync.dma_start(rec_l.rearrange("(a p) n -> p a n", p=128), rec_sb["l"][:])
        nc.sync.dma_start(rec_r.rearrange("(a p) n -> p a n", p=128), rec_sb["r"][:])

    nc.compile()
    return nc


_NC_CACHE = {}


def _get_nc():
    if "nc" not in _NC_CACHE:
        _NC_CACHE["nc"] = _build_nc()
    return _NC_CACHE["nc"]


# ---------------------------------------------------------------------------
# Host side


def _expected_edge_index():
    local = np.arange(L)
    s = np.repeat(local, L)
    d = np.tile(local, L)
    m = s != d
    src = np.concatenate([s[m], local, L + local, np.arange(N)])
    dst = np.concatenate([d[m], L + local, local, np.arange(N)])
    return np.stack([src, dst])


def _np_reference_fallback(node_features, gat, norm, dec_local, dec_remote,
                           node_types, edge_index):
    """Pure-numpy replica of the reference; used only if the edge structure
    is not the expected deterministic pattern."""
    x = node_features @ gat["W"]
    xh = x.reshape(N, HEADS, HID)
    a_src = (xh * gat["att_src"]).sum(-1)
    a_dst = (xh * gat["att_dst"]).sum(-1)
    src, dst = edge_index[0], edge_index[1]
    e = a_src[src] + a_dst[dst]
    e = np.where(e >= 0, e, NEG_SLOPE * e).astype(np.float32)
    emax = np.full((N, HEADS), -np.inf, np.float32)
    np.maximum.at(emax, dst, e)
    ex = np.exp(e - emax[dst])
    zs = np.zeros((N, HEADS), np.float32)
    np.add.at(zs, dst, ex)
    alpha = ex / zs[dst]
    msg = xh[src] * alpha[:, :, None]
    out = np.zeros((N, HEADS, HID), np.float32)
    np.add.at(out, dst, msg)
    out = out.reshape(N, OUT) + gat["bias"]

    def ln(v, g, b):
        m = v.mean(-1, keepdims=True)
        var = ((v - m) ** 2).mean(-1, keepdims=True)
        return (v - m) / np.sqrt(var + EPS) * g + b

    h = ln(out, norm["g"], norm["b"])

    def dec(v, p):
        t = ln(np.maximum(v @ p["W1"] + p["b1"], 0), p["g1"], p["b1n"])
        t = ln(np.maximum(t @ p["W2"] + p["b2"], 0), p["g2"], p["b2n"])
        return t @ p["W3"] + p["b3"]

    rl = dec(h, dec_local)
    rr = dec(h, dec_remote)
    rec = np.where((node_types == 1)[:, None], rl, rr)
    return rec.astype(np.float32), edge_index, alpha.astype(np.float32)


def _to_np(v):
    return {k: _to_np(x) for k, x in v.items()} if isinstance(v, dict) else np.asarray(v)


def kernel(node_features, gat, norm, dec_local, dec_remote, node_types,
           edge_index, trace=False):
    _ensure_axon_hooks_stub()
    node_features = np.asarray(node_features, np.float32)
    gat, norm = _to_np(gat), _to_np(norm)
    dec_local, dec_remote = _to_np(dec_local), _to_np(dec_remote)
    node_types_np = np.asarray(node_types)
    edge_index_np = np.asarray(edge_index)

    if not np.array_equal(edge_index_np.astype(np.int64), _expected_edge_index()):
        return _np_reference_fallback(
            node_features, gat, norm, dec_local, dec_remote,
            node_types_np, edge_index_np,
        )

    from concourse.bass_utils import run_bass_kernel_spmd

    f32 = np.float32
    W = gat["W"].astype(f32)
    att_src = gat["att_src"].astype(f32)
    att_dst = gat["att_dst"].astype(f32)
    Wsrc = (W.reshape(MAPPED, HEADS, HID) * att_src[None]).sum(-1)
    Wdst = (W.reshape(MAPPED, HEADS, HID) * att_dst[None]).sum(-1)
    Wg_ext = np.ascontiguousarray(np.concatenate([W, Wsrc, Wdst], axis=1))
    nfT = np.ascontiguousarray(node_features.T)

    import ml_dtypes
    bf = ml_dtypes.bfloat16

    # fold LN affine params into the following linear layer; weights in bf16,
    # biases pre-broadcast to [128, w] f32
    def fold(dec):
        g, b = norm["g"].astype(f32), norm["b"].astype(f32)
        W1 = g[:, None] * dec["W1"]
        b1 = dec["b1"] + b @ dec["W1"]
        W2 = dec["g1"][:, None] * dec["W2"]
        b2 = dec["b2"] + dec["b1n"] @ dec["W2"]
        W3 = dec["g2"][:, None] * dec["W3"]
        b3 = dec["b3"] + dec["b2n"] @ dec["W3"]

        def bc(v):
            return np.ascontiguousarray(
                np.broadcast_to(v.reshape(1, -1).astype(f32), (128, v.size))
            )

        return (
            np.ascontiguousarray(W1.astype(bf)), bc(b1),
            np.ascontiguousarray(W2.astype(bf)), bc(b2),
            np.ascontiguousarray(W3.astype(bf)), bc(b3),
        )

    W1l, b1l, W2l, b2l, W3l, b3l = fold(dec_local)
    W1r, b1r, W2r, b2r, W3r, b3r = fold(dec_remote)

    shared = {
        "nfT": nfT, "Wg": Wg_ext,
        "bgat": gat["bias"].reshape(1, -1).astype(f32),
        "W1l": W1l, "b1l": b1l, "W2l": W2l, "b2l": b2l, "W3l": W3l, "b3l": b3l,
        "W1r": W1r, "b1r": b1r, "W2r": W2r, "b2r": b2r, "W3r": W3r, "b3r": b3r,
    }
    in_maps = []
    for c in range(NCORES):
        m = dict(shared)
        m["nfT_loc"] = np.ascontiguousarray(nfT[:, c * SH : (c + 1) * SH])
        m["nfT_rem"] = np.ascontiguousarray(nfT[:, L + c * SH : L + (c + 1) * SH])
        in_maps.append(m)

    nc = _get_nc()
    res = run_bass_kernel_spmd(nc, in_maps, core_ids=list(range(NCORES)), trace=trace)
    _NC_CACHE["last_results"] = res
    outs = res.results

    # ---- host unshard / assembly ----
    # alphaT_full[s, d, h] = exp(e) * R
    alphaT_full = np.empty((L, L, HEADS), np.float32)
    a_rl = np.empty((L, HEADS), np.float32)
    a_lr = np.empty((L, HEADS), np.float32)
    a_rr = np.empty((L, HEADS), np.float32)
    for c in range(NCORES):
        sl = slice(c * SH, (c + 1) * SH)
        sm = outs[c]["asmall"]
        a_rl[sl], a_lr[sl], a_rr[sl] = sm[:, 0:4], sm[:, 4:8], sm[:, 8:12]
        R = sm[:, 12:16]  # [d, h]
        blk = outs[c]["alpha_t"].reshape(128, HEADS, NLOC_T, SH)  # [p, h, t, d]
        # -> [s=t*128+p, d, h]
        blk = np.transpose(blk, (2, 0, 3, 1)).reshape(L, SH, HEADS)
        alphaT_full[:, sl, :] = blk * R[None, :, :]

    per_edge = alphaT_full.reshape(L * L, HEADS)
    mask = ~np.eye(L, dtype=bool).reshape(-1)
    clique = per_edge[mask]
    idx = np.arange(L)
    self_local = alphaT_full[idx, idx, :]
    alpha = np.concatenate(
        [clique, a_lr, a_rl, self_local, a_rr], axis=0
    ).astype(np.float32)

    rec_local = np.empty((N, IN_DIM), np.float32)
    rec_remote = np.empty((N, IN_DIM), np.float32)
    for c in range(NCORES):
        sl = slice(c * SH, (c + 1) * SH)
        slr = slice(L + c * SH, L + (c + 1) * SH)
        rec_local[sl] = outs[c]["rec_l"][0:SH]
        rec_local[slr] = outs[c]["rec_l"][SH : 2 * SH]
        rec_remote[sl] = outs[c]["rec_r"][0:SH]
        rec_remote[slr] = outs[c]["rec_r"][SH : 2 * SH]
    reconstructed = np.where((node_types_np == 1)[:, None], rec_local, rec_remote)

    return reconstructed.astype(np.float32), edge_index_np, alpha
